# revision 1
# baseline (speedup 1.0000x reference)
"""Trainium2 Bass kernel for nn_NNModel2 (2x NNConv GNN + pooled MLP readout).

Self-contained: accepts FULL inputs, shards edges across 8 NeuronCores
(edge-parallel, node-aligned ownership by dst), runs one SPMD Bass program,
returns the FULL [256, 1] output.

Math (per NNConv layer, aggr='add'):
    w_e  = (edge_attr @ nn_w + nn_b).reshape(E, I, O)
    msg  = einsum('ei,eio->eo', x[src], w_e)
    out  = segment_sum(msg, dst, N) + x @ root_w + bias
restructured as one dense matmul over z:
    z[e, (k,i)] = edge_attr[e,k] * x[src[e], i]
    msg = z @ W' + x[src] @ B';  W'[(k,i), o] = nn_w[k, i*O+o]
Scatter-add and graph pooling are one-hot matmuls (is_equal vs iota consts).
conv1 -> AllGather h1 (bf16) -> conv2 -> pooled partials -> AllReduce -> MLP.
"""

import sys

sys.path.insert(0, "/opt/trn_rl_repo")

import numpy as np

from concourse import bacc, bass, mybir
import concourse.tile as tile
from concourse import bass_utils

P = 128
NCORES = 8
N_NODES = 4096
N_EDGES = 8192
N_GRAPHS = 256
DN = 64
DE = 32
H = 256
NSH = N_NODES // NCORES  # 512
NT = NSH // P  # 4
GT = N_GRAPHS // P  # 2

F32 = mybir.dt.float32
BF16 = mybir.dt.bfloat16
I16 = mybir.dt.int16
AF = mybir.ActivationFunctionType
ALU = mybir.AluOpType

_cache = {}


def _wrap_idx(idx, n):
    idx = np.asarray(idx, dtype=np.int16)
    assert idx.shape == (n,) and n % 16 == 0
    return np.tile(idx.reshape(n // 16, 16).T, (8, 1)).copy()


def _build(e_pad, upto="full"):
    ET = e_pad // P
    nc = bacc.Bacc(num_devices=NCORES)

    x = nc.dram_tensor("x", [N_NODES, DN], F32, kind="ExternalInput")
    attr = nc.dram_tensor("attr", [N_EDGES, DE], F32, kind="ExternalInput")
    nn1_w = nc.dram_tensor("nn1_w", [DE, DN * H], F32, kind="ExternalInput")
    nn1_b = nc.dram_tensor("nn1_b", [1, DN * H], F32, kind="ExternalInput")
    r1w = nc.dram_tensor("r1w", [DN, H], F32, kind="ExternalInput")
    b1 = nc.dram_tensor("b1", [1, H], F32, kind="ExternalInput")
    nn2_w = nc.dram_tensor("nn2_w", [DE, H * H], F32, kind="ExternalInput")
    nn2_b = nc.dram_tensor("nn2_b", [1, H * H], F32, kind="ExternalInput")
    r2w = nc.dram_tensor("r2w", [H, H], F32, kind="ExternalInput")
    b2 = nc.dram_tensor("b2", [1, H], F32, kind="ExternalInput")
    l1w = nc.dram_tensor("l1w", [H, H // 2], F32, kind="ExternalInput")
    l1b = nc.dram_tensor("l1b", [H // 2, 1], F32, kind="ExternalInput")
    l2w = nc.dram_tensor("l2w", [H // 2, 1], F32, kind="ExternalInput")
    l2b = nc.dram_tensor("l2b", [1, 1], F32, kind="ExternalInput")
    src_w = nc.dram_tensor("src_w", [P, e_pad // 16], I16, kind="ExternalInput")
    eid_w = nc.dram_tensor("eid_w", [P, e_pad // 16], I16, kind="ExternalInput")
    node_w = nc.dram_tensor("node_w", [P, NSH // 16], I16, kind="ExternalInput")
    dstl = nc.dram_tensor("dstl", [e_pad, 1], F32, kind="ExternalInput")
    batchl = nc.dram_tensor("batchl", [NSH, 1], F32, kind="ExternalInput")
    iota512 = nc.dram_tensor("iota512", [P, NSH], F32, kind="ExternalInput")
    iotag = nc.dram_tensor("iotag", [P, N_GRAPHS], F32, kind="ExternalInput")
    ident = nc.dram_tensor("ident", [P, P], F32, kind="ExternalInput")
    out = nc.dram_tensor("out", [N_GRAPHS, 1], F32, kind="ExternalOutput")

    def dbg_out(name, shape):
        return nc.dram_tensor(name, shape, F32, kind="ExternalOutput")

    rg = [list(range(NCORES))]
    ST = {"w": 1, "gather": 1, "msg1": 2, "h1": 2, "ag": 3, "h2": 4, "full": 99}[upto]

    with tile.TileContext(nc, num_cores=NCORES) as tc:
        with (
            tc.tile_pool(name="const", bufs=1) as cp,
            tc.tile_pool(name="work", bufs=3) as wp,
            tc.tile_pool(name="dram", bufs=1, space="DRAM") as dr,
        ):
            # ======== stage 0: resident weights (bf16) + bf16 DRAM tables
            w2sb = cp.tile([P, 2 * DE, H], BF16)
            w2_src = nn2_w.rearrange("k (h p o) -> p (k h) o", h=2, p=P, o=H)
            with tc.tile_pool(name="staging", bufs=2) as stp:
                w1sb = cp.tile([P, 16, H], BF16)
                w1_src = nn1_w.rearrange("(t k2) (i o) -> (k2 i) t o", k2=2, o=H)
                for c in range(2):
                    st1 = stp.tile([P, 8, H], F32, tag="w2st", name=f"w1st{c}")
                    nc.sync.dma_start(out=st1[:], in_=w1_src[:, 8 * c : 8 * (c + 1), :])
                    nc.scalar.activation(
                        out=w1sb[:, 8 * c : 8 * (c + 1), :], in_=st1[:], func=AF.Copy
                    )

                def load_bf(dst_tile, src_ap, tag="bst"):
                    sst = stp.tile(
                        list(src_ap.shape), F32, tag=tag,
                        name=f"st_{dst_tile.tensor.name}",
                    )
                    nc.sync.dma_start(out=sst[:], in_=src_ap)
                    nc.vector.tensor_copy(out=dst_tile[:], in_=sst[:])

                b1p = cp.tile([DN, H], BF16)
                load_bf(b1p, nn1_b.rearrange("one (i o) -> (one i) o", o=H))
                b2p = cp.tile([P, 2, H], BF16)
                load_bf(b2p, nn2_b.rearrange("one (h p o) -> (one p) h o", h=2, p=P, o=H))
                r1wb = cp.tile([DN, H], BF16)
                load_bf(r1wb, r1w[:])
                r2wb = cp.tile([P, 2, H], BF16)
                load_bf(r2wb, r2w.rearrange("(h p) o -> p h o", p=P))
                l1wb = cp.tile([P, 2, H // 2], BF16)
                load_bf(l1wb, l1w.rearrange("(h p) m -> p h m", p=P))
                l2wb = cp.tile([H // 2, 1], BF16)
                load_bf(l2wb, l2w[:], tag="bst2")
                identb = cp.tile([P, P], BF16)
                load_bf(identb, ident[:])

                b1sb = cp.tile([1, H], F32)
                nc.sync.dma_start(out=b1sb[:], in_=b1[:])
                b2sb = cp.tile([1, H], F32)
                nc.sync.dma_start(out=b2sb[:], in_=b2[:])
                l1bsb = cp.tile([H // 2, 1], F32)
                nc.sync.dma_start(out=l1bsb[:], in_=l1b[:])
                l2bsb = cp.tile([1, 1], F32)
                nc.sync.dma_start(out=l2bsb[:], in_=l2b[:])
                ones = cp.tile([1, P], F32)
                nc.vector.memset(ones[:], 1.0)
                io512 = cp.tile([P, NSH], F32)
                nc.sync.dma_start(out=io512[:], in_=iota512[:])
                iog = cp.tile([P, N_GRAPHS], F32)
                nc.sync.dma_start(out=iog[:], in_=iotag[:])
                dstl_sb = cp.tile([P, ET, 1], F32)
                nc.sync.dma_start(
                    out=dstl_sb[:], in_=dstl.rearrange("(e p) one -> p e one", p=P)
                )
                batchl_sb = cp.tile([P, NT, 1], F32)
                nc.sync.dma_start(
                    out=batchl_sb[:], in_=batchl.rearrange("(t p) one -> p t one", p=P)
                )
                srcw_sb = cp.tile([P, e_pad // 16], I16)
                nc.sync.dma_start(out=srcw_sb[:], in_=src_w[:])
                eidw_sb = cp.tile([P, e_pad // 16], I16)
                nc.sync.dma_start(out=eidw_sb[:], in_=eid_w[:])
                nodew_sb = cp.tile([P, NSH // 16], I16)
                nc.sync.dma_start(out=nodew_sb[:], in_=node_w[:])

                x_dup = dr.tile([N_NODES, P], BF16)
                stx = stp.tile([P, N_NODES // P, DN], F32, tag="xst", bufs=1)
                nc.sync.dma_start(out=stx[:], in_=x.rearrange("(nb p) d -> p nb d", p=P))
                xbf = stp.tile([P, N_NODES // P, DN], BF16, tag="xbf", bufs=1)
                nc.vector.tensor_copy(out=xbf[:], in_=stx[:])
                x_dup_v = x_dup[:].rearrange("(nb p) c -> p nb c", p=P)
                nc.sync.dma_start(out=x_dup_v[:, :, 0:DN], in_=xbf[:])
                nc.sync.dma_start(out=x_dup_v[:, :, DN : 2 * DN], in_=xbf[:])

                attr_pad = dr.tile([N_EDGES, P], BF16)
                sta = stp.tile([P, N_EDGES // P, DE], F32, tag="xst", bufs=1)
                nc.sync.dma_start(
                    out=sta[:], in_=attr.rearrange("(nb p) d -> p nb d", p=P)
                )
                apd = stp.tile([P, N_EDGES // P, DE], BF16, tag="apd", bufs=1)
                nc.vector.tensor_copy(out=apd[:], in_=sta[:])
                nc.sync.dma_start(
                    out=attr_pad[:].rearrange("(nb p) c -> p nb c", p=P)[:, :, 0:DE],
                    in_=apd[:],
                )

                # W2 last: only needed at conv2; let gather-chain DMAs go first
                for c in range(8):
                    st = stp.tile([P, 8, H], F32, tag="w2st", name=f"w2st{c}")
                    nc.sync.dma_start(out=st[:], in_=w2_src[:, 8 * c : 8 * (c + 1), :])
                    nc.scalar.activation(
                        out=w2sb[:, 8 * c : 8 * (c + 1), :], in_=st[:], func=AF.Copy
                    )

            # ======== stage 1: gathers + attr broadcast tiles
            with tc.tile_pool(name="big", bufs=1) as bp:
                attrT = cp.tile([P, 1, e_pad], BF16)
                nc.gpsimd.dma_gather(
                    out_ap=attrT[:], in_ap=attr_pad[:], idxs_ap=eidw_sb[:],
                    num_idxs=e_pad, num_idxs_reg=e_pad, elem_size=P, transpose=True, single_packet=False,
                )
                attrT_dram = dr.tile([DE, e_pad], BF16)
                nc.sync.dma_start(out=attrT_dram[:], in_=attrT[0:DE, 0, :])

                xsrcT = cp.tile([P, 1, e_pad], BF16)
                nc.gpsimd.dma_gather(
                    out_ap=xsrcT[:], in_ap=x_dup[:], idxs_ap=srcw_sb[:],
                    num_idxs=e_pad, num_idxs_reg=e_pad, elem_size=P, transpose=True, single_packet=False,
                )
                xshT = cp.tile([P, 1, NSH], BF16)
                nc.gpsimd.dma_gather(
                    out_ap=xshT[:], in_ap=x_dup[:], idxs_ap=nodew_sb[:],
                    num_idxs=NSH, num_idxs_reg=NSH, elem_size=P, transpose=True, single_packet=False,
                )

                bc_all = bp.tile([P, DE, e_pad], BF16, name="bc_all")
                for kc in range(4):
                    nc.sync.dma_start(
                        out=bc_all[:, 8 * kc : 8 * (kc + 1), :],
                        in_=attrT_dram[8 * kc : 8 * (kc + 1), :].partition_broadcast(P),
                    )

                if upto == "w":
                    dw1 = dbg_out("d_w1", [P, 16 * H])
                    for j in range(2):
                        tw = wp.tile([P, 8, H], F32, tag="dbgw")
                        nc.vector.tensor_copy(out=tw[:], in_=w1sb[:, 8*j:8*(j+1), :])
                        nc.sync.dma_start(
                            out=dw1[:].rearrange("p (t o) -> p t o", o=H)[:, 8*j:8*(j+1), :],
                            in_=tw[:])
                    dw2 = dbg_out("d_w2", [P, 4 * H])
                    tw2 = wp.tile([P, 4, H], F32, tag="dbgw2")
                    nc.vector.tensor_copy(out=tw2[:], in_=w2sb[:, 0:4, :])
                    nc.sync.dma_start(
                        out=dw2[:].rearrange("p (t o) -> p t o", o=H), in_=tw2[:])

                if ST == 1 and upto == "gather":
                    d1 = dbg_out("d_xsrcT", [P, e_pad])
                    tmp = wp.tile([P, e_pad], F32, tag="dbgf")
                    nc.vector.tensor_copy(out=tmp[:], in_=xsrcT[:, 0, :])
                    nc.sync.dma_start(out=d1[:], in_=tmp[:])
                    d2 = dbg_out("d_attrT", [DE, e_pad])
                    tmp2 = wp.tile([DE, e_pad], F32, tag="dbg2")
                    nc.vector.tensor_copy(out=tmp2[:], in_=attrT[0:DE, 0, :])
                    nc.sync.dma_start(out=d2[:], in_=tmp2[:])
                    d3 = dbg_out("d_bc5", [P, e_pad])
                    tmp3 = wp.tile([P, e_pad], F32, tag="dbgf")
                    nc.vector.tensor_copy(out=tmp3[:], in_=bc_all[:, 5, :])
                    nc.sync.dma_start(out=d3[:], in_=tmp3[:])

                if ST >= 2:
                    with tc.tile_pool(name="psA", bufs=1, space="PSUM") as psA:
                        # ======== stage 2: conv1
                        msg_ps = [
                            psA.tile([P, 2 * H], F32, space="PSUM",
                                     tag=f"msg{j}", name=f"msg1_{j}")
                            for j in range((ET + 1) // 2)
                        ]

                        def m1(e):
                            return msg_ps[e // 2][:, (e % 2) * H : (e % 2) * H + H]

                        for t in range(16):
                            k0, k1 = 2 * t, 2 * t + 1
                            zt = wp.tile([P, e_pad], BF16, tag="zt", bufs=4)
                            nc.vector.tensor_tensor(
                                out=zt[0:DN, :], in0=xsrcT[0:DN, 0, :],
                                in1=bc_all[0:DN, k0, :], op=ALU.mult,
                            )
                            nc.vector.tensor_tensor(
                                out=zt[DN:P, :], in0=xsrcT[DN:P, 0, :],
                                in1=bc_all[DN:P, k1, :], op=ALU.mult,
                            )
                            for e in range(ET):
                                nc.tensor.matmul(
                                    m1(e), lhsT=zt[:, P * e : P * (e + 1)],
                                    rhs=w1sb[:, t, :],
                                    start=(t == 0 and e % 2 == 0), stop=False,
                                    skip_group_check=True,
                                )
                        for e in range(ET):
                            nc.tensor.matmul(
                                m1(e), lhsT=xsrcT[0:DN, 0, P * e : P * (e + 1)],
                                rhs=b1p[:], start=False, stop=True,
                                skip_group_check=True,
                            )

                        if upto == "msg1":
                            dz = dbg_out("d_z0", [P, e_pad])
                            zt0 = wp.tile([P, e_pad], BF16, tag="zt")
                            nc.vector.tensor_tensor(
                                out=zt0[0:DN, :], in0=xsrcT[0:DN, 0, :],
                                in1=bc_all[0:DN, 0, :], op=ALU.mult)
                            nc.vector.tensor_tensor(
                                out=zt0[DN:P, :], in0=xsrcT[DN:P, 0, :],
                                in1=bc_all[DN:P, 1, :], op=ALU.mult)
                            tmpz = wp.tile([P, e_pad], F32, tag="dbgf")
                            nc.vector.tensor_copy(out=tmpz[:], in_=zt0[:])
                            nc.sync.dma_start(out=dz[:], in_=tmpz[:])
                            dm = dbg_out("d_msg1", [P, ET * H])
                            for j in range((ET + 1) // 2):
                                w = min(2 * H, (ET - 2 * j) * H)
                                tmpm = wp.tile([P, 2 * H], F32, tag="dbgm")
                                nc.scalar.activation(
                                    out=tmpm[:, 0:w], in_=msg_ps[j][:, 0:w],
                                    func=AF.Copy)
                                nc.sync.dma_start(
                                    out=dm[:, 2 * H * j : 2 * H * j + w],
                                    in_=tmpm[:, 0:w])

                        agg_ps = [
                            psA.tile([P, 2 * H], F32, space="PSUM",
                                     tag=f"agg{j}", name=f"agg1_{j}")
                            for j in range(NT // 2)
                        ]

                        def a1(n):
                            return agg_ps[n // 2][:, (n % 2) * H : (n % 2) * H + H]

                        msbs = []
                        for j in range((ET + 1) // 2) if upto != "msg1" else []:
                            w = min(2 * H, (ET - 2 * j) * H)
                            msb = wp.tile([P, 2 * H], BF16, tag="msb")
                            nc.scalar.activation(
                                out=msb[:, 0:w], in_=msg_ps[j][:, 0:w], func=AF.Copy
                            )
                            msbs.append(msb)
                        for e in range(ET) if upto != "msg1" else []:
                            for n in range(NT):
                                oh = wp.tile([P, P], BF16, tag="oh", bufs=6)
                                nc.vector.tensor_scalar(
                                    out=oh[:], in0=io512[:, P * n : P * (n + 1)],
                                    scalar1=dstl_sb[:, e, :1], scalar2=None,
                                    op0=ALU.is_equal,
                                )
                                nc.tensor.matmul(
                                    a1(n), lhsT=oh[:],
                                    rhs=msbs[e // 2][:, (e % 2) * H : (e % 2) * H + H],
                                    start=(e == 0 and n % 2 == 0), stop=False,
                                    skip_group_check=True,
                                )
                        for n in range(NT) if upto != "msg1" else []:
                            nc.tensor.matmul(
                                a1(n), lhsT=xshT[0:DN, 0, P * n : P * (n + 1)],
                                rhs=r1wb[:], start=False, stop=False,
                                skip_group_check=True,
                            )
                            nc.tensor.matmul(
                                a1(n), lhsT=ones[:], rhs=b1sb[:],
                                start=False, stop=True, skip_group_check=True,
                            )
                        h1sb = bp.tile([P, NT, H], BF16)
                        for j in range(NT // 2) if upto != "msg1" else []:
                            nc.scalar.activation(
                                out=h1sb[:, 2 * j : 2 * j + 2, :],
                                in_=agg_ps[j][:, 0 : 2 * H], func=AF.Relu,
                            )

                        if ST == 2 and upto == "h1":
                            dh = dbg_out("d_h1", [P, NT * H])
                            tmp = wp.tile([P, NT, H], F32, tag="dbgf")
                            nc.vector.tensor_copy(out=tmp[:], in_=h1sb[:])
                            nc.sync.dma_start(
                                out=dh[:].rearrange("p (t o) -> p t o", o=H),
                                in_=tmp[:],
                            )

                        if ST >= 3:
                            h1cc = dr.tile([NSH, H], BF16)
                            nc.sync.dma_start(
                                out=h1cc[:].rearrange("(t p) o -> p t o", p=P),
                                in_=h1sb[:],
                            )
                            h1_all = dr.tile([N_NODES, H], BF16, addr_space="Shared")
                            nc.gpsimd.collective_compute(
                                "AllGather", ALU.bypass, replica_groups=rg,
                                ins=[h1cc[:].opt()], outs=[h1_all[:].opt()],
                            )
                        if ST == 3:
                            dh = dbg_out("d_h1all", [P, (N_NODES // P) * H])
                            stg = bp.tile([P, N_NODES // P, H], BF16)
                            nc.sync.dma_start(
                                out=stg[:],
                                in_=h1_all[:].rearrange("(nb p) o -> p nb o", p=P),
                            )
                            for nb in range(N_NODES // P):
                                tmpg = wp.tile([P, H], F32, tag="dbgf")
                                nc.vector.tensor_copy(out=tmpg[:], in_=stg[:, nb, :])
                                nc.sync.dma_start(
                                    out=dh[:, H * nb : H * (nb + 1)], in_=tmpg[:]
                                )

                        if ST >= 4:
                            # ======== stage 3+4: conv2
                            h1srcT = bp.tile([P, 2, e_pad], BF16)
                            nc.gpsimd.dma_gather(
                                out_ap=h1srcT[:], in_ap=h1_all[:], idxs_ap=srcw_sb[:],
                                num_idxs=e_pad, num_idxs_reg=e_pad, elem_size=H,
                                transpose=True, single_packet=False,
                            )
                            h1shT = bp.tile([P, 2, NSH], BF16)
                            nc.gpsimd.dma_gather(
                                out_ap=h1shT[:], in_ap=h1_all[:], idxs_ap=nodew_sb[:],
                                num_idxs=NSH, num_idxs_reg=NSH, elem_size=H,
                                transpose=True, single_packet=False,
                            )

                            msg2_ps = [
                                psA.tile([P, 2 * H], F32, space="PSUM",
                                         tag=f"msg{j}", name=f"msg2_{j}")
                                for j in range((ET + 1) // 2)
                            ]

                            def m2(e):
                                return msg2_ps[e // 2][:, (e % 2) * H : (e % 2) * H + H]

                            for t in range(64):
                                k, ih = t // 2, t % 2
                                zt = wp.tile([P, e_pad], BF16, tag="zt", bufs=4)
                                nc.vector.tensor_tensor(
                                    out=zt[:], in0=h1srcT[:, ih, :], in1=bc_all[:, k, :],
                                    op=ALU.mult,
                                )
                                for e in range(ET):
                                    nc.tensor.matmul(
                                        m2(e), lhsT=zt[:, P * e : P * (e + 1)],
                                        rhs=w2sb[:, t, :],
                                        start=(t == 0 and e % 2 == 0), stop=False,
                                        skip_group_check=True,
                                    )
                            for e in range(ET):
                                for ih in range(2):
                                    nc.tensor.matmul(
                                        m2(e),
                                        lhsT=h1srcT[:, ih, P * e : P * (e + 1)],
                                        rhs=b2p[:, ih, :], start=False,
                                        stop=(ih == 1), skip_group_check=True,
                                    )

                            agg2_ps = [
                                psA.tile([P, 2 * H], F32, space="PSUM",
                                         tag=f"agg{j}", name=f"agg2_{j}")
                                for j in range(NT // 2)
                            ]

                            def a2(n):
                                return agg2_ps[n // 2][:, (n % 2) * H : (n % 2) * H + H]

                            msbs2 = []
                            for j in range((ET + 1) // 2):
                                w = min(2 * H, (ET - 2 * j) * H)
                                msb = wp.tile([P, 2 * H], BF16, tag="msb")
                                nc.scalar.activation(
                                    out=msb[:, 0:w], in_=msg2_ps[j][:, 0:w],
                                    func=AF.Copy,
                                )
                                msbs2.append(msb)
                            for e in range(ET):
                                for n in range(NT):
                                    oh = wp.tile([P, P], BF16, tag="oh", bufs=6)
                                    nc.vector.tensor_scalar(
                                        out=oh[:], in0=io512[:, P * n : P * (n + 1)],
                                        scalar1=dstl_sb[:, e, :1], scalar2=None,
                                        op0=ALU.is_equal,
                                    )
                                    nc.tensor.matmul(
                                        a2(n), lhsT=oh[:],
                                        rhs=msbs2[e // 2][:, (e % 2) * H : (e % 2) * H + H],
                                        start=(e == 0 and n % 2 == 0), stop=False,
                                        skip_group_check=True,
                                    )
                            for n in range(NT):
                                for kh in range(2):
                                    nc.tensor.matmul(
                                        a2(n),
                                        lhsT=h1shT[:, kh, P * n : P * (n + 1)],
                                        rhs=r2wb[:, kh, :], start=False, stop=False,
                                        skip_group_check=True,
                                    )
                                nc.tensor.matmul(
                                    a2(n), lhsT=ones[:], rhs=b2sb[:],
                                    start=False, stop=True, skip_group_check=True,
                                )
                            h2e = bp.tile([P, NT, H + 1], BF16)
                            nc.vector.memset(h2e[:, :, H : H + 1], 1.0)
                            for j in range(NT // 2):
                                nc.scalar.activation(
                                    out=h2e[:, 2 * j : 2 * j + 2, 0:H],
                                    in_=agg2_ps[j][:, 0 : 2 * H], func=AF.Copy,
                                )

                        if ST == 4:
                            dh = dbg_out("d_h2", [P, NT * H])
                            tmp = wp.tile([P, NT, H], F32, tag="dbgf")
                            for n in range(NT):
                                nc.vector.tensor_copy(
                                    out=tmp[:, n, :], in_=h2e[:, n, 0:H]
                                )
                            nc.sync.dma_start(
                                out=dh[:].rearrange("p (t o) -> p t o", o=H),
                                in_=tmp[:],
                            )

                        if ST >= 5:
                            # ======== stage 5: pooling
                            pool_ps = [
                                psA.tile([P, 2 * H], F32, space="PSUM",
                                         tag=f"agg{g}", name=f"pool_{g}")
                                for g in range(GT)
                            ]
                            for n in range(NT):
                                for g in range(GT):
                                    ohg = wp.tile([P, P], BF16, tag="oh", bufs=6)
                                    nc.vector.tensor_scalar(
                                        out=ohg[:], in0=iog[:, P * g : P * (g + 1)],
                                        scalar1=batchl_sb[:, n, :1], scalar2=None,
                                        op0=ALU.is_equal,
                                    )
                                    nc.tensor.matmul(
                                        pool_ps[g][:, 0 : H + 1], lhsT=ohg[:],
                                        rhs=h2e[:, n, :], start=(n == 0),
                                        stop=(n == NT - 1),
                                        skip_group_check=(n not in (0, NT - 1)),
                                    )
                            plsb = bp.tile([P, GT, H + 1], F32)
                            for g in range(GT):
                                nc.scalar.activation(
                                    out=plsb[:, g, :], in_=pool_ps[g][:, 0 : H + 1],
                                    func=AF.Copy,
                                )
                            pcc_in = dr.tile([N_GRAPHS, H + 1], F32)
                            nc.sync.dma_start(
                                out=pcc_in[:].rearrange("(g p) c -> p g c", p=P),
                                in_=plsb[:],
                            )
                            pcc_out = dr.tile([N_GRAPHS, H + 1], F32, addr_space="Shared")
                            nc.gpsimd.collective_compute(
                                "AllReduce", ALU.add, replica_groups=rg,
                                ins=[pcc_in[:].opt()], outs=[pcc_out[:].opt()],
                            )

                if ST >= 5:
                    # ======== stage 6: readout MLP (every core, redundant)
                    with tc.tile_pool(name="psB", bufs=1, space="PSUM") as psB:
                        pl = bp.tile([P, GT, H + 1], F32)
                        nc.sync.dma_start(
                            out=pl[:],
                            in_=pcc_out[:].rearrange("(g p) c -> p g c", p=P),
                        )
                        rec = bp.tile([P, GT, 1], F32)
                        cnt = wp.tile([P, GT, 1], F32, tag="cnt")
                        nc.vector.tensor_scalar_max(cnt[:], pl[:, :, H : H + 1], 1.0)
                        for g in range(GT):
                            nc.vector.reciprocal(out=rec[:, g, :], in_=cnt[:, g, :])
                        mean_bf = bp.tile([P, GT, H], BF16)
                        for g in range(GT):
                            nc.vector.tensor_scalar(
                                out=mean_bf[:, g, :], in0=pl[:, g, 0:H],
                                scalar1=rec[:, g, :1], scalar2=None, op0=ALU.mult,
                            )
                        poolT = bp.tile([P, 2, N_GRAPHS], BF16)
                        for g in range(GT):
                            for hh in range(2):
                                tp = psB.tile([P, P], BF16, space="PSUM", tag="tp")
                                nc.tensor.transpose(
                                    out=tp[:],
                                    in_=mean_bf[:, g, P * hh : P * (hh + 1)],
                                    identity=identb[:],
                                )
                                nc.scalar.activation(
                                    out=poolT[:, hh, P * g : P * (g + 1)],
                                    in_=tp[:], func=AF.Copy,
                                )
                        z1_ps = psB.tile([P, N_GRAPHS], F32, space="PSUM", tag="z1")
                        for kh in range(2):
                            nc.tensor.matmul(
                                z1_ps[:], lhsT=l1wb[:, kh, :], rhs=poolT[:, kh, :],
                                start=(kh == 0), stop=(kh == 1),
                            )
                        z1sb = bp.tile([P, N_GRAPHS], BF16)
                        nc.scalar.activation(
                            out=z1sb[:], in_=z1_ps[:], func=AF.Relu, bias=l1bsb[:, :1]
                        )
                        o_ps = psB.tile([1, N_GRAPHS], F32, space="PSUM", tag="op")
                        nc.tensor.matmul(
                            o_ps[:], lhsT=l2wb[:], rhs=z1sb[:], start=True, stop=True
                        )
                        osb = bp.tile([1, N_GRAPHS], F32)
                        nc.scalar.activation(
                            out=osb[:], in_=o_ps[:], func=AF.Sigmoid, bias=l2bsb[:, :1]
                        )
                        nc.sync.dma_start(
                            out=out[:].rearrange("g one -> one g"), in_=osb[:]
                        )

    nc.compile()
    return nc


def _prep_inputs(inputs, e_pad=None):
    x = np.asarray(inputs["x"], dtype=np.float32)
    ei = np.asarray(inputs["edge_index"])
    attr = np.asarray(inputs["edge_attr"], dtype=np.float32)
    batch = np.asarray(inputs["batch"])
    src, dst = ei[0].astype(np.int64), ei[1].astype(np.int64)

    owner = dst // NSH
    per_core = [np.nonzero(owner == c)[0] for c in range(NCORES)]
    need = max(max(len(e) for e in per_core), 1)
    if e_pad is None:
        e_pad = max(((need + P - 1) // P) * P, P)
    assert need <= e_pad

    common = {
        "x": x,
        "attr": attr,
        "nn1_w": np.asarray(inputs["nn1_w"], dtype=np.float32),
        "nn1_b": np.asarray(inputs["nn1_b"], dtype=np.float32).reshape(1, -1),
        "r1w": np.asarray(inputs["root1_w"], dtype=np.float32),
        "b1": np.asarray(inputs["bias1"], dtype=np.float32).reshape(1, -1),
        "nn2_w": np.asarray(inputs["nn2_w"], dtype=np.float32),
        "nn2_b": np.asarray(inputs["nn2_b"], dtype=np.float32).reshape(1, -1),
        "r2w": np.asarray(inputs["root2_w"], dtype=np.float32),
        "b2": np.asarray(inputs["bias2"], dtype=np.float32).reshape(1, -1),
        "l1w": np.asarray(inputs["lin1_w"], dtype=np.float32),
        "l1b": np.asarray(inputs["lin1_b"], dtype=np.float32).reshape(-1, 1),
        "l2w": np.asarray(inputs["lin2_w"], dtype=np.float32),
        "l2b": np.asarray(inputs["lin2_b"], dtype=np.float32).reshape(1, 1),
        "iota512": np.tile(np.arange(NSH, dtype=np.float32), (P, 1)),
        "iotag": np.tile(np.arange(N_GRAPHS, dtype=np.float32), (P, 1)),
        "ident": np.eye(P, dtype=np.float32),
    }

    in_maps = []
    for c in range(NCORES):
        eids = per_core[c]
        ne = len(eids)
        src_c = np.zeros(e_pad, dtype=np.int16)
        src_c[:ne] = src[eids]
        eid_c = np.zeros(e_pad, dtype=np.int16)
        eid_c[:ne] = eids
        dstl_c = np.full(e_pad, -1.0, dtype=np.float32)
        dstl_c[:ne] = (dst[eids] - c * NSH).astype(np.float32)
        node_c = np.arange(c * NSH, (c + 1) * NSH, dtype=np.int16)
        batch_c = batch[c * NSH : (c + 1) * NSH].astype(np.float32)
        m = dict(common)
        m["src_w"] = _wrap_idx(src_c, e_pad)
        m["eid_w"] = _wrap_idx(eid_c, e_pad)
        m["node_w"] = _wrap_idx(node_c, NSH)
        m["dstl"] = dstl_c.reshape(-1, 1)
        m["batchl"] = batch_c.reshape(-1, 1)
        in_maps.append(m)
    return e_pad, in_maps


def kernel(**inputs) -> np.ndarray:
    e_pad, in_maps = _prep_inputs(inputs)
    if e_pad not in _cache:
        _cache[e_pad] = _build(e_pad)
    nc = _cache[e_pad]
    res = bass_utils.run_bass_kernel_spmd(nc, in_maps, core_ids=list(range(NCORES)))
    return np.asarray(res.results[0]["out"], dtype=np.float32)


def run_debug(upto, **inputs):
    e_pad, in_maps = _prep_inputs(inputs)
    nc = _build(e_pad, upto=upto)
    res = bass_utils.run_bass_kernel_spmd(nc, in_maps, core_ids=list(range(NCORES)))
    return e_pad, res



# revision 9
# speedup vs baseline: 1.6157x; 1.6157x over previous
"""Trainium2 Bass kernel for nn_NNModel2 (2x NNConv GNN + pooled MLP readout).

Self-contained: accepts FULL inputs, returns the FULL [256, 1] output.

Sharding: one balanced node-ownership map (512 nodes/core, chosen so that both
per-core edge counts are ~1024). conv1 uses dst-sharded edges (x is replicated
so x[src] is a local gather); conv2 uses src-sharded edges so h1[src] is a
local gather. conv2's output feeds only the linear graph mean-pool, so its
messages are scattered directly into per-graph partial sums; one bf16
AllReduce [256,256] combines them. No AllGather anywhere.

Math (per NNConv layer, aggr='add'):
    w_e  = (edge_attr @ nn_w + nn_b).reshape(E, I, O)
    msg  = einsum('ei,eio->eo', x[src], w_e)
restructured as a dense matmul over z[e,(k,i)] = attr[e,k] * x[src[e],i]:
    msg = z @ W' + x[src] @ B'
Scatter-add (nodes for conv1, graphs for conv2) via one-hot matmuls.
"""

import sys

sys.path.insert(0, "/opt/trn_rl_repo")

import numpy as np

from concourse import bacc, bass, mybir
import concourse.tile as tile
from concourse import bass_utils

P = 128
NCORES = 8
N_NODES = 4096
N_EDGES = 8192
N_GRAPHS = 256
DN = 64
DE = 32
H = 256
NSH = N_NODES // NCORES  # 512
NT = NSH // P  # 4
GT = N_GRAPHS // P  # 2

F32 = mybir.dt.float32
BF16 = mybir.dt.bfloat16
F16 = mybir.dt.float16
I16 = mybir.dt.int16
AF = mybir.ActivationFunctionType
ALU = mybir.AluOpType

_cache = {}


def _wrap_idx(idx, n):
    idx = np.asarray(idx, dtype=np.int16)
    assert idx.shape == (n,) and n % 16 == 0
    return np.tile(idx.reshape(n // 16, 16).T, (8, 1)).copy()


def _build(e_padA, e_padB, upto="full"):
    ETA = e_padA // P
    ETB = e_padB // P
    nc = bacc.Bacc(num_devices=NCORES)

    # ---- external inputs (host-prepped layouts, bf16 where possible)
    x_dup = nc.dram_tensor("x_dup", [N_NODES, P], BF16, kind="ExternalInput")
    attr_pad = nc.dram_tensor("attr_pad", [N_EDGES, P], BF16, kind="ExternalInput")
    w1p = nc.dram_tensor("w1p", [P, 16, H], BF16, kind="ExternalInput")
    b1p_in = nc.dram_tensor("b1p_in", [DN, H], BF16, kind="ExternalInput")
    r1w_in = nc.dram_tensor("r1w_in", [DN, H], BF16, kind="ExternalInput")
    b1row_in = nc.dram_tensor("b1row_in", [1, H], BF16, kind="ExternalInput")
    w2p = nc.dram_tensor("w2p", [P, 64, H], BF16, kind="ExternalInput")
    b2p_in = nc.dram_tensor("b2p_in", [P, 2, H], BF16, kind="ExternalInput")
    r2w_in = nc.dram_tensor("r2w_in", [P, 2, H], BF16, kind="ExternalInput")
    b2colT_in = nc.dram_tensor("b2colT_in", [P, 2, 1], BF16, kind="ExternalInput")
    l1w_in = nc.dram_tensor("l1w_in", [P, 2, H // 2], BF16, kind="ExternalInput")
    l1b_in = nc.dram_tensor("l1b_in", [H // 2, 1], F32, kind="ExternalInput")
    l2w_in = nc.dram_tensor("l2w_in", [H // 2, 1], BF16, kind="ExternalInput")
    l2b_in = nc.dram_tensor("l2b_in", [1, 1], F32, kind="ExternalInput")
    io512_in = nc.dram_tensor("io512_in", [P, NSH], F16, kind="ExternalInput")
    iotag_in = nc.dram_tensor("iotag_in", [P, N_GRAPHS], F16, kind="ExternalInput")
    ident_in = nc.dram_tensor("ident_in", [P, P], BF16, kind="ExternalInput")
    recl_in = nc.dram_tensor("recl_in", [N_GRAPHS, 1], F32, kind="ExternalInput")
    mask_in = nc.dram_tensor("mask_in", [1, N_GRAPHS], BF16, kind="ExternalInput")
    # per-core index tables
    srcA_w = nc.dram_tensor("srcA_w", [P, e_padA // 16], I16, kind="ExternalInput")
    eidA_w = nc.dram_tensor("eidA_w", [P, e_padA // 16], I16, kind="ExternalInput")
    ownA_w = nc.dram_tensor("ownA_w", [P, NSH // 16], I16, kind="ExternalInput")
    srcB_w = nc.dram_tensor("srcB_w", [P, e_padB // 16], I16, kind="ExternalInput")
    eidB_w = nc.dram_tensor("eidB_w", [P, e_padB // 16], I16, kind="ExternalInput")
    id512_w = nc.dram_tensor("id512_w", [P, NSH // 16], I16, kind="ExternalInput")
    dstlA_in = nc.dram_tensor("dstlA_in", [e_padA, 1], F32, kind="ExternalInput")
    gdstB_in = nc.dram_tensor("gdstB_in", [e_padB, 1], F32, kind="ExternalInput")
    batchl_in = nc.dram_tensor("batchl_in", [NSH, 1], F32, kind="ExternalInput")
    out = nc.dram_tensor("out", [N_GRAPHS, 1], F32, kind="ExternalOutput")

    def dbg_out(name, shape):
        return nc.dram_tensor(name, shape, F32, kind="ExternalOutput")

    rg = [list(range(NCORES))]
    ST = {"gather": 1, "msg1": 2, "h1": 3, "msg2": 4, "poolmsg": 5, "pool": 5, "ohg": 5, "full": 99}[upto]

    with tile.TileContext(nc, num_cores=NCORES) as tc:
        with (
            tc.tile_pool(name="const", bufs=1) as cp,
            tc.tile_pool(name="work", bufs=3) as wp,
            tc.tile_pool(name="dram", bufs=1, space="DRAM") as dr,
        ):
            # ======== stage 0: index tables first (gathers depend on them)
            srcA_sb = cp.tile([P, e_padA // 16], I16)
            nc.sync.dma_start(out=srcA_sb[:], in_=srcA_w[:])
            eidA_sb = cp.tile([P, e_padA // 16], I16)
            nc.sync.dma_start(out=eidA_sb[:], in_=eidA_w[:])
            ownA_sb = cp.tile([P, NSH // 16], I16)
            nc.sync.dma_start(out=ownA_sb[:], in_=ownA_w[:])
            srcB_sb = cp.tile([P, e_padB // 16], I16)
            nc.sync.dma_start(out=srcB_sb[:], in_=srcB_w[:])
            eidB_sb = cp.tile([P, e_padB // 16], I16)
            nc.sync.dma_start(out=eidB_sb[:], in_=eidB_w[:])
            id512_sb = cp.tile([P, NSH // 16], I16)
            nc.sync.dma_start(out=id512_sb[:], in_=id512_w[:])

            # ---- gathers on Pool queue
            xsrcT = cp.tile([P, 1, e_padA], BF16)
            nc.gpsimd.dma_gather(
                out_ap=xsrcT[:], in_ap=x_dup[:], idxs_ap=srcA_sb[:],
                num_idxs=e_padA, num_idxs_reg=e_padA, elem_size=P,
                transpose=True, single_packet=False,
            )
            attrTA = cp.tile([P, 1, e_padA], BF16)
            nc.gpsimd.dma_gather(
                out_ap=attrTA[:], in_ap=attr_pad[:], idxs_ap=eidA_sb[:],
                num_idxs=e_padA, num_idxs_reg=e_padA, elem_size=P,
                transpose=True, single_packet=False,
            )
            attrTA_dram = dr.tile([DE, e_padA], BF16)
            nc.sync.dma_start(out=attrTA_dram[:], in_=attrTA[0:DE, 0, :])
            attrTB = cp.tile([P, 1, e_padB], BF16)
            nc.gpsimd.dma_gather(
                out_ap=attrTB[:], in_ap=attr_pad[:], idxs_ap=eidB_sb[:],
                num_idxs=e_padB, num_idxs_reg=e_padB, elem_size=P,
                transpose=True, single_packet=False,
            )
            attrTB_dram = dr.tile([DE, e_padB], BF16)
            nc.sync.dma_start(out=attrTB_dram[:], in_=attrTB[0:DE, 0, :])
            xshT = cp.tile([P, 1, NSH], BF16)
            nc.gpsimd.dma_gather(
                out_ap=xshT[:], in_ap=x_dup[:], idxs_ap=ownA_sb[:],
                num_idxs=NSH, num_idxs_reg=NSH, elem_size=P,
                transpose=True, single_packet=False,
            )

            # ---- conv1 broadcast (pair layout) + w1, then conv2 streams
            bcA = cp.tile([P, 16, e_padA], BF16)
            for c in range(2):
                sl = slice(8 * c, 8 * (c + 1))
                nc.sync.dma_start(
                    out=bcA[0:DN, sl, :],
                    in_=attrTA_dram[16 * c : 16 * (c + 1) : 2, :].partition_broadcast(DN),
                )
                nc.sync.dma_start(
                    out=bcA[DN:P, sl, :],
                    in_=attrTA_dram[16 * c + 1 : 16 * (c + 1) : 2, :].partition_broadcast(DN),
                )
            w1sb = cp.tile([P, 16, H], BF16)
            nc.sync.dma_start(out=w1sb[:], in_=w1p[:])
            b1p = cp.tile([DN, H], BF16)
            nc.sync.dma_start(out=b1p[:], in_=b1p_in[:])
            r1wb = cp.tile([DN, H], BF16)
            nc.sync.dma_start(out=r1wb[:], in_=r1w_in[:])
            b1row = cp.tile([1, H], BF16)
            nc.sync.dma_start(out=b1row[:], in_=b1row_in[:])
            io512 = cp.tile([P, NSH], F16)
            nc.sync.dma_start(out=io512[:], in_=io512_in[:])
            iotag = cp.tile([P, N_GRAPHS], F16)
            nc.sync.dma_start(out=iotag[:], in_=iotag_in[:])
            dstlA_sb = cp.tile([P, ETA, 1], F32)
            nc.sync.dma_start(
                out=dstlA_sb[:], in_=dstlA_in.rearrange("(e p) one -> p e one", p=P)
            )
            gdstl_sb = cp.tile([P, ETB, 1], F32)
            nc.sync.dma_start(
                out=gdstl_sb[:], in_=gdstB_in.rearrange("(e p) one -> p e one", p=P)
            )
            batchl_sb = cp.tile([P, NT, 1], F32)
            nc.sync.dma_start(
                out=batchl_sb[:], in_=batchl_in.rearrange("(t p) one -> p t one", p=P)
            )
            ones = cp.tile([1, P], BF16)
            nc.vector.memset(ones[:], 1.0)

            # ---- conv2 weights + broadcast: stream during conv1
            w2sb = cp.tile([P, 64, H], BF16)
            bcB = cp.tile([P, DE, e_padB], BF16)
            nc.sync.dma_start(out=w2sb[:, 0:16, :], in_=w2p[:, 0:16, :])
            nc.sync.dma_start(
                out=bcB[:, 0:8, :],
                in_=attrTB_dram[0:8, :].partition_broadcast(P),
            )
            for c in range(1, 4):
                nc.sync.dma_start(
                    out=w2sb[:, 16 * c : 16 * (c + 1), :],
                    in_=w2p[:, 16 * c : 16 * (c + 1), :],
                )
                nc.sync.dma_start(
                    out=bcB[:, 8 * c : 8 * (c + 1), :],
                    in_=attrTB_dram[8 * c : 8 * (c + 1), :].partition_broadcast(P),
                )
            # small consts for tail
            r2wb = cp.tile([P, 2, H], BF16)
            nc.sync.dma_start(out=r2wb[:], in_=r2w_in[:])
            b2p = cp.tile([P, 2, H], BF16)
            nc.sync.dma_start(out=b2p[:], in_=b2p_in[:])
            b2colT = cp.tile([P, 2, 1], BF16)
            nc.sync.dma_start(out=b2colT[:], in_=b2colT_in[:])
            l1wb = cp.tile([P, 2, H // 2], BF16)
            nc.sync.dma_start(out=l1wb[:], in_=l1w_in[:])
            l1bsb = cp.tile([H // 2, 1], F32)
            nc.sync.dma_start(out=l1bsb[:], in_=l1b_in[:])
            l2wb = cp.tile([H // 2, 1], BF16)
            nc.sync.dma_start(out=l2wb[:], in_=l2w_in[:])
            l2bsb = cp.tile([1, 1], F32)
            nc.sync.dma_start(out=l2bsb[:], in_=l2b_in[:])
            identb = cp.tile([P, P], BF16)
            nc.sync.dma_start(out=identb[:], in_=ident_in[:])
            recl = cp.tile([P, GT, 1], F32)
            nc.sync.dma_start(
                out=recl[:], in_=recl_in.rearrange("(g p) one -> p g one", p=P)
            )
            masksb = cp.tile([1, N_GRAPHS], BF16)
            nc.sync.dma_start(out=masksb[:], in_=mask_in[:])

            if upto == "gather":
                for nm, t_ in (("d_xsrcT", xsrcT), ("d_attrTB", attrTB)):
                    d = dbg_out(nm, [P, t_.tensor.shape[2]])
                    tmp = wp.tile([P, t_.tensor.shape[2]], F32, tag="dbgf")
                    nc.vector.tensor_copy(out=tmp[:], in_=t_[:, 0, :])
                    nc.sync.dma_start(out=d[:], in_=tmp[:])
                d = dbg_out("d_bcB5", [P, e_padB])
                tmp = wp.tile([P, e_padB], F32, tag="dbgf")
                nc.vector.tensor_copy(out=tmp[:], in_=bcB[:, 5, :])
                nc.sync.dma_start(out=d[:], in_=tmp[:])

            with tc.tile_pool(name="psA", bufs=1, space="PSUM") as psA:
                # ======== conv1: msg1 = z1 @ W1' + x_src @ B1'
                msg_ps = [
                    psA.tile([P, 2 * H], F32, space="PSUM",
                             tag=f"msg{j}", name=f"msg1_{j}")
                    for j in range((ETA + 1) // 2)
                ]

                def m1(e):
                    return msg_ps[e // 2][:, (e % 2) * H : (e % 2) * H + H]

                if ST >= 2:
                    for t in range(16):
                        eng = nc.vector if t % 2 == 0 else nc.gpsimd
                        zt = wp.tile([P, e_padA], BF16, tag="zt", bufs=4)
                        eng.tensor_tensor(
                            out=zt[:], in0=xsrcT[:, 0, :], in1=bcA[:, t, :],
                            op=ALU.mult,
                        )
                        for e in range(ETA):
                            nc.tensor.matmul(
                                m1(e), lhsT=zt[:, P * e : P * (e + 1)],
                                rhs=w1sb[:, t, :],
                                start=(t == 0 and e % 2 == 0), stop=False,
                                skip_group_check=True,
                            )
                    for e in range(ETA):
                        nc.tensor.matmul(
                            m1(e), lhsT=xsrcT[0:DN, 0, P * e : P * (e + 1)],
                            rhs=b1p[:], start=False, stop=True,
                            skip_group_check=True,
                        )

                    msbs = []
                    for j in range((ETA + 1) // 2):
                        w = min(2 * H, (ETA - 2 * j) * H)
                        msb = wp.tile([P, 2 * H], BF16, tag="msb")
                        nc.scalar.activation(
                            out=msb[:, 0:w], in_=msg_ps[j][:, 0:w], func=AF.Copy
                        )
                        msbs.append(msb)

                if upto == "msg1":
                    dm = dbg_out("d_msg1", [P, ETA * H])
                    for j in range((ETA + 1) // 2):
                        w = min(2 * H, (ETA - 2 * j) * H)
                        tmpm = wp.tile([P, 2 * H], F32, tag="dbgm")
                        nc.vector.tensor_copy(out=tmpm[:, 0:w], in_=msbs[j][:, 0:w])
                        nc.sync.dma_start(
                            out=dm[:, 2 * H * j : 2 * H * j + w], in_=tmpm[:, 0:w])

                # ---- conv1 scatter to own nodes + root + bias, relu
                agg_ps = [
                    psA.tile([P, 2 * H], F32, space="PSUM",
                             tag=f"agg{j}", name=f"agg1_{j}")
                    for j in range(NT // 2)
                ]

                def a1(n):
                    return agg_ps[n // 2][:, (n % 2) * H : (n % 2) * H + H]

                if ST >= 3:
                    for e in range(ETA):
                        for n in range(NT):
                            oh = wp.tile([P, P], BF16, tag="oh", bufs=6)
                            nc.vector.tensor_scalar(
                                out=oh[:], in0=io512[:, P * n : P * (n + 1)],
                                scalar1=dstlA_sb[:, e, :1], scalar2=None,
                                op0=ALU.is_equal,
                            )
                            nc.tensor.matmul(
                                a1(n), lhsT=oh[:],
                                rhs=msbs[e // 2][:, (e % 2) * H : (e % 2) * H + H],
                                start=(e == 0 and n % 2 == 0), stop=False,
                                skip_group_check=True,
                            )
                    for n in range(NT):
                        nc.tensor.matmul(
                            a1(n), lhsT=xshT[0:DN, 0, P * n : P * (n + 1)],
                            rhs=r1wb[:], start=False, stop=False,
                            skip_group_check=True,
                        )
                        nc.tensor.matmul(
                            a1(n), lhsT=ones[:], rhs=b1row[:],
                            start=False, stop=True, skip_group_check=True,
                        )
                    h1sb = cp.tile([P, NT, H], BF16)
                    for j in range(NT // 2):
                        nc.scalar.activation(
                            out=h1sb[:, 2 * j : 2 * j + 2, :],
                            in_=agg_ps[j][:, 0 : 2 * H], func=AF.Relu,
                        )
                    h1loc = dr.tile([NSH, H], BF16)
                    nc.sync.dma_start(
                        out=h1loc[:].rearrange("(t p) o -> p t o", p=P), in_=h1sb[:]
                    )

                    # local gathers for conv2
                    h1srcT = cp.tile([P, 2, e_padB], BF16)
                    nc.gpsimd.dma_gather(
                        out_ap=h1srcT[:], in_ap=h1loc[:], idxs_ap=srcB_sb[:],
                        num_idxs=e_padB, num_idxs_reg=e_padB, elem_size=H,
                        transpose=True, single_packet=False,
                    )
                    h1ownT = cp.tile([P, 2, NSH], BF16)
                    nc.gpsimd.dma_gather(
                        out_ap=h1ownT[:], in_ap=h1loc[:], idxs_ap=id512_sb[:],
                        num_idxs=NSH, num_idxs_reg=NSH, elem_size=H,
                        transpose=True, single_packet=False,
                    )

                if upto == "h1":
                    dh = dbg_out("d_h1", [P, NT * H])
                    tmp = wp.tile([P, NT, H], F32, tag="dbgf")
                    nc.vector.tensor_copy(out=tmp[:], in_=h1sb[:])
                    nc.sync.dma_start(
                        out=dh[:].rearrange("p (t o) -> p t o", o=H), in_=tmp[:]
                    )

                # ---- early small matmul: b2l1 = b2 @ l1w (for bias2 folding)
                with tc.tile_pool(name="psS", bufs=1, space="PSUM") as psS:
                    b2l1_ps = psS.tile([1, H // 2], F32, space="PSUM", tag="b2l1")
                    for kh in range(2):
                        nc.tensor.matmul(
                            b2l1_ps[:], lhsT=b2colT[:, kh, :1], rhs=l1wb[:, kh, :],
                            start=(kh == 0), stop=(kh == 1),
                        )
                    b2l1sb = cp.tile([1, H // 2], BF16)
                    nc.scalar.activation(out=b2l1sb[:], in_=b2l1_ps[:], func=AF.Copy)

                # ======== conv2: msg2 = z2 @ W2' + h1_src @ B2'
                if ST >= 4:
                    msg2_ps = [
                        psA.tile([P, 2 * H], F32, space="PSUM",
                                 tag=f"msg{j}", name=f"msg2_{j}")
                        for j in range((ETB + 1) // 2)
                    ]

                    def m2(e):
                        return msg2_ps[e // 2][:, (e % 2) * H : (e % 2) * H + H]

                    for t in range(64):
                        k, ih = t // 2, t % 2
                        eng = nc.vector if t % 2 == 0 else nc.gpsimd
                        zt = wp.tile([P, e_padB], BF16, tag="zt", bufs=4)
                        eng.tensor_tensor(
                            out=zt[:], in0=h1srcT[:, ih, :], in1=bcB[:, k, :],
                            op=ALU.mult,
                        )
                        for e in range(ETB):
                            nc.tensor.matmul(
                                m2(e), lhsT=zt[:, P * e : P * (e + 1)],
                                rhs=w2sb[:, t, :],
                                start=(t == 0 and e % 2 == 0), stop=False,
                                skip_group_check=True,
                            )
                    for e in range(ETB):
                        for ih in range(2):
                            nc.tensor.matmul(
                                m2(e), lhsT=h1srcT[:, ih, P * e : P * (e + 1)],
                                rhs=b2p[:, ih, :], start=False,
                                stop=(ih == 1), skip_group_check=True,
                            )

                    msbs2 = []
                    for j in range((ETB + 1) // 2):
                        w = min(2 * H, (ETB - 2 * j) * H)
                        msb = wp.tile([P, 2 * H], BF16, tag="msb")
                        nc.scalar.activation(
                            out=msb[:, 0:w], in_=msg2_ps[j][:, 0:w], func=AF.Copy
                        )
                        msbs2.append(msb)

                if upto == "msg2":
                    dm = dbg_out("d_msg2", [P, ETB * H])
                    for j in range((ETB + 1) // 2):
                        w = min(2 * H, (ETB - 2 * j) * H)
                        tmpm = wp.tile([P, 2 * H], F32, tag="dbgm")
                        nc.vector.tensor_copy(out=tmpm[:, 0:w], in_=msbs2[j][:, 0:w])
                        nc.sync.dma_start(
                            out=dm[:, 2 * H * j : 2 * H * j + w], in_=tmpm[:, 0:w])

                # ---- conv2 root transform r2 = h1_own @ root2_w
                if ST >= 4:
                    r2_ps = [
                        psA.tile([P, 2 * H], F32, space="PSUM",
                                 tag=f"agg{j}", name=f"r2_{j}")
                        for j in range(NT // 2)
                    ]
                    for n in range(NT):
                        for kh in range(2):
                            nc.tensor.matmul(
                                r2_ps[n // 2][:, (n % 2) * H : (n % 2) * H + H],
                                lhsT=h1ownT[:, kh, P * n : P * (n + 1)],
                                rhs=r2wb[:, kh, :],
                                start=(kh == 0), stop=(kh == 1),
                                skip_group_check=True,
                            )
                    r2sb = cp.tile([P, NT, H], BF16)
                    for j in range(NT // 2):
                        nc.scalar.activation(
                            out=r2sb[:, 2 * j : 2 * j + 2, :],
                            in_=r2_ps[j][:, 0 : 2 * H], func=AF.Copy,
                        )

                    # ---- scatter msg2 by graph(dst), r2 by graph(node)
                    pool_ps_t = psA.tile([P, 2 * H], F32, space="PSUM",
                                         tag="pool", name="pool")
                    pool_ps = [pool_ps_t[:, g * H : (g + 1) * H] for g in range(GT)]
                    for e in range(ETB):
                        for g in range(GT):
                            ohg = wp.tile([P, P], BF16, tag="oh", bufs=6)
                            nc.vector.tensor_scalar(
                                out=ohg[:], in0=iotag[:, P * g : P * (g + 1)],
                                scalar1=gdstl_sb[:, e, :1], scalar2=None,
                                op0=ALU.is_equal,
                            )
                            if upto == "ohg" and e <= 1 and g == 0:
                                doh = dbg_out(f"d_ohg{e}", [P, P])
                                tmpo = wp.tile([P, P], F32, tag="dbgo")
                                nc.vector.tensor_copy(out=tmpo[:], in_=ohg[:])
                                nc.sync.dma_start(out=doh[:], in_=tmpo[:])
                            nc.tensor.matmul(
                                pool_ps[g], lhsT=ohg[:],
                                rhs=msbs2[e // 2][:, (e % 2) * H : (e % 2) * H + H],
                                start=(e == 0 and g == 0),
                                stop=(upto == "poolmsg" and e == ETB - 1),
                                skip_group_check=True,
                            )
                    for n in range(NT if upto != "poolmsg" else 0):
                        for g in range(GT):
                            ohb = wp.tile([P, P], BF16, tag="oh", bufs=6)
                            nc.vector.tensor_scalar(
                                out=ohb[:], in0=iotag[:, P * g : P * (g + 1)],
                                scalar1=batchl_sb[:, n, :1], scalar2=None,
                                op0=ALU.is_equal,
                            )
                            nc.tensor.matmul(
                                pool_ps[g], lhsT=ohb[:],
                                rhs=r2sb[:, n, :],
                                start=False, stop=(n == NT - 1),
                                skip_group_check=True,
                            )
                    plsb = cp.tile([P, GT, H], BF16)
                    for g in range(GT):
                        nc.scalar.activation(
                            out=plsb[:, g, :], in_=pool_ps[g], func=AF.Copy
                        )
                    pcc_in = dr.tile([N_GRAPHS, H], BF16)
                    nc.sync.dma_start(
                        out=pcc_in[:].rearrange("(g p) c -> p g c", p=P), in_=plsb[:]
                    )
                    pcc_out = dr.tile([N_GRAPHS, H], BF16, addr_space="Shared")
                    nc.gpsimd.collective_compute(
                        "AllReduce", ALU.add, replica_groups=rg,
                        ins=[pcc_in[:].opt()], outs=[pcc_out[:].opt()],
                    )

                if upto == "ohg":
                    dgd = dbg_out("d_gdstl", [P, ETB])
                    tmpg = wp.tile([P, ETB, 1], F32, tag="dbgg")
                    nc.vector.tensor_copy(out=tmpg[:], in_=gdstl_sb[:])
                    nc.sync.dma_start(
                        out=dgd[:].rearrange("p e -> p e ()"), in_=tmpg[:])
                if upto == "poolmsg":
                    dpm = dbg_out("d_pool", [P, GT * H])
                    tmpm_ = wp.tile([P, GT, H], F32, tag="dbgf")
                    nc.vector.tensor_copy(out=tmpm_[:], in_=plsb[:])
                    nc.sync.dma_start(
                        out=dpm[:].rearrange("p (g o) -> p g o", o=H), in_=tmpm_[:]
                    )
                if upto == "pool":
                    dr2 = dbg_out("d_r2", [P, NT * H])
                    tmp2 = wp.tile([P, NT, H], F32, tag="dbgr")
                    nc.vector.tensor_copy(out=tmp2[:], in_=r2sb[:])
                    nc.sync.dma_start(
                        out=dr2[:].rearrange("p (t o) -> p t o", o=H), in_=tmp2[:]
                    )
                    dp = dbg_out("d_pool", [P, GT * H])
                    tmp = wp.tile([P, GT, H], F32, tag="dbgf")
                    nc.vector.tensor_copy(out=tmp[:], in_=plsb[:])
                    nc.sync.dma_start(
                        out=dp[:].rearrange("p (g o) -> p g o", o=H), in_=tmp[:]
                    )

            # ======== tail: readout MLP (redundant on every core)
            if ST >= 5 and upto == "full":
                with tc.tile_pool(name="psB", bufs=1, space="PSUM") as psB:
                    pl = cp.tile([P, GT, H], BF16)
                    nc.sync.dma_start(
                        out=pl[:], in_=pcc_out[:].rearrange("(g p) c -> p g c", p=P)
                    )
                    mean_bf = cp.tile([P, GT, H], BF16)
                    for g in range(GT):
                        nc.vector.tensor_scalar(
                            out=mean_bf[:, g, :], in0=pl[:, g, :],
                            scalar1=recl[:, g, :1], scalar2=None, op0=ALU.mult,
                        )
                    poolT = cp.tile([P, 2, N_GRAPHS], BF16)
                    for g in range(GT):
                        for hh in range(2):
                            tp = psB.tile([P, P], BF16, space="PSUM", tag="tp")
                            nc.tensor.transpose(
                                out=tp[:],
                                in_=mean_bf[:, g, P * hh : P * (hh + 1)],
                                identity=identb[:],
                            )
                            nc.scalar.activation(
                                out=poolT[:, hh, P * g : P * (g + 1)],
                                in_=tp[:], func=AF.Copy,
                            )
                    z1_ps = psB.tile([P, N_GRAPHS], F32, space="PSUM", tag="z1")
                    for kh in range(2):
                        nc.tensor.matmul(
                            z1_ps[:], lhsT=l1wb[:, kh, :], rhs=poolT[:, kh, :],
                            start=(kh == 0), stop=False, skip_group_check=True,
                        )
                    nc.tensor.matmul(
                        z1_ps[:], lhsT=b2l1sb[:], rhs=masksb[:],
                        start=False, stop=True, skip_group_check=True,
                    )
                    z1sb = cp.tile([P, N_GRAPHS], BF16)
                    nc.scalar.activation(
                        out=z1sb[:], in_=z1_ps[:], func=AF.Relu, bias=l1bsb[:, :1]
                    )
                    o_ps = psB.tile([1, N_GRAPHS], F32, space="PSUM", tag="op")
                    nc.tensor.matmul(
                        o_ps[:], lhsT=l2wb[:], rhs=z1sb[:], start=True, stop=True
                    )
                    osb = cp.tile([1, N_GRAPHS], F32)
                    nc.scalar.activation(
                        out=osb[:], in_=o_ps[:], func=AF.Sigmoid, bias=l2bsb[:, :1]
                    )
                    nc.sync.dma_start(
                        out=out[:].rearrange("g one -> one g"), in_=osb[:]
                    )

    nc.compile()
    return nc


def _balance_owner(src, dst):
    """Assign 512 nodes/core s.t. per-core indeg and outdeg sums are balanced."""
    indeg = np.bincount(dst, minlength=N_NODES)
    outdeg = np.bincount(src, minlength=N_NODES)
    order = np.argsort(-(indeg + outdeg), kind="stable")
    owner = np.full(N_NODES, -1, dtype=np.int64)
    in_load = np.zeros(NCORES, dtype=np.int64)
    out_load = np.zeros(NCORES, dtype=np.int64)
    slots = np.full(NCORES, NSH, dtype=np.int64)
    for n in order:
        best, bkey = -1, None
        for c in range(NCORES):
            if slots[c] == 0:
                continue
            key = (
                max(in_load[c] + indeg[n], out_load[c] + outdeg[n]),
                in_load[c] + out_load[c],
            )
            if bkey is None or key < bkey:
                best, bkey = c, key
        owner[n] = best
        in_load[best] += indeg[n]
        out_load[best] += outdeg[n]
        slots[best] -= 1
    return owner, int(in_load.max()), int(out_load.max())


def _bf16(a):
    import ml_dtypes

    return np.asarray(np.asarray(a, np.float32), dtype=ml_dtypes.bfloat16)


def _prep_inputs(inputs):
    x = np.asarray(inputs["x"], dtype=np.float32)
    ei = np.asarray(inputs["edge_index"])
    attr = np.asarray(inputs["edge_attr"], dtype=np.float32)
    batch = np.asarray(inputs["batch"]).astype(np.int64)
    src, dst = ei[0].astype(np.int64), ei[1].astype(np.int64)

    owner, max_in, max_out = _balance_owner(src, dst)
    e_padA = max(((max_in + P - 1) // P) * P, P)
    e_padB = max(((max_out + P - 1) // P) * P, P)

    # own node lists (ascending global id) and local ids
    own = [np.nonzero(owner == c)[0] for c in range(NCORES)]
    local_id = np.zeros(N_NODES, dtype=np.int64)
    for c in range(NCORES):
        local_id[own[c]] = np.arange(NSH)

    nn1_w = np.asarray(inputs["nn1_w"], dtype=np.float32)  # [32, 64*256]
    nn2_w = np.asarray(inputs["nn2_w"], dtype=np.float32)  # [32, 256*256]
    nn1_b = np.asarray(inputs["nn1_b"], dtype=np.float32)
    nn2_b = np.asarray(inputs["nn2_b"], dtype=np.float32)

    # w1p[p, t, o] = nn1_w[2t + p//64, (p%64)*256 + o]
    w1r = nn1_w.reshape(16, 2, DN, H)  # [t, k2, i, o]
    w1p = np.ascontiguousarray(w1r.transpose(1, 2, 0, 3).reshape(P, 16, H))
    # w2p[p, t, o] = nn2_w[t//2, ((t%2)*128 + p)*256 + o]
    w2r = nn2_w.reshape(DE, 2, P, H)  # [k, ih, p, o]
    w2p = np.ascontiguousarray(w2r.transpose(2, 0, 1, 3).reshape(P, 64, H))
    b2pr = nn2_b.reshape(2, P, H).transpose(1, 0, 2)  # [p, ih, o]

    cnt = np.bincount(batch, minlength=N_GRAPHS).astype(np.float32)
    recl = (1.0 / np.maximum(cnt, 1.0)).reshape(N_GRAPHS, 1).astype(np.float32)
    maskrow = (cnt > 0).astype(np.float32).reshape(1, N_GRAPHS)

    r2w = np.asarray(inputs["root2_w"], dtype=np.float32)  # [256, 256]
    b2 = np.asarray(inputs["bias2"], dtype=np.float32)  # [256]
    l1w = np.asarray(inputs["lin1_w"], dtype=np.float32)  # [256, 128]

    common = {
        "x_dup": _bf16(np.concatenate([x, x], axis=1)),
        "attr_pad": _bf16(
            np.concatenate(
                [attr, np.zeros((N_EDGES, P - DE), np.float32)], axis=1
            )
        ),
        "w1p": _bf16(w1p),
        "b1p_in": _bf16(nn1_b.reshape(DN, H)),
        "r1w_in": _bf16(np.asarray(inputs["root1_w"], np.float32)),
        "b1row_in": _bf16(np.asarray(inputs["bias1"], np.float32).reshape(1, H)),
        "w2p": _bf16(w2p),
        "b2p_in": _bf16(b2pr),
        "r2w_in": _bf16(r2w.reshape(2, P, H).transpose(1, 0, 2)),
        "b2colT_in": _bf16(b2.reshape(2, P, 1).transpose(1, 0, 2)),
        "l1w_in": _bf16(l1w.reshape(2, P, H // 2).transpose(1, 0, 2)),
        "l1b_in": np.asarray(inputs["lin1_b"], np.float32).reshape(-1, 1),
        "l2w_in": _bf16(np.asarray(inputs["lin2_w"], np.float32)),
        "l2b_in": np.asarray(inputs["lin2_b"], np.float32).reshape(1, 1),
        "io512_in": np.tile(np.arange(NSH, dtype=np.float16), (P, 1)),
        "iotag_in": np.tile(np.arange(N_GRAPHS, dtype=np.float16), (P, 1)),
        "ident_in": _bf16(np.eye(P, dtype=np.float32)),
        "recl_in": recl,
        "mask_in": _bf16(maskrow),
    }

    in_maps = []
    for c in range(NCORES):
        eA = np.nonzero(owner[dst] == c)[0]
        eB = np.nonzero(owner[src] == c)[0]
        nA, nB = len(eA), len(eB)
        assert nA <= e_padA and nB <= e_padB

        srcA = np.zeros(e_padA, dtype=np.int16)
        srcA[:nA] = src[eA]
        eidA = np.zeros(e_padA, dtype=np.int16)
        eidA[:nA] = eA
        dstlA = np.full(e_padA, -1.0, dtype=np.float32)
        dstlA[:nA] = local_id[dst[eA]].astype(np.float32)

        srcB = np.zeros(e_padB, dtype=np.int16)
        srcB[:nB] = local_id[src[eB]]
        eidB = np.zeros(e_padB, dtype=np.int16)
        eidB[:nB] = eB
        gdstB = np.full(e_padB, -1.0, dtype=np.float32)
        gdstB[:nB] = batch[dst[eB]].astype(np.float32)

        m = dict(common)
        m["srcA_w"] = _wrap_idx(srcA, e_padA)
        m["eidA_w"] = _wrap_idx(eidA, e_padA)
        m["ownA_w"] = _wrap_idx(own[c].astype(np.int16), NSH)
        m["srcB_w"] = _wrap_idx(srcB, e_padB)
        m["eidB_w"] = _wrap_idx(eidB, e_padB)
        m["id512_w"] = _wrap_idx(np.arange(NSH, dtype=np.int16), NSH)
        m["dstlA_in"] = dstlA.reshape(-1, 1)
        m["gdstB_in"] = gdstB.reshape(-1, 1)
        m["batchl_in"] = batch[own[c]].astype(np.float32).reshape(-1, 1)
        in_maps.append(m)
    return (e_padA, e_padB), in_maps


def kernel(**inputs) -> np.ndarray:
    key, in_maps = _prep_inputs(inputs)
    if key not in _cache:
        _cache[key] = _build(*key)
    nc = _cache[key]
    res = bass_utils.run_bass_kernel_spmd(nc, in_maps, core_ids=list(range(NCORES)))
    return np.asarray(res.results[0]["out"], dtype=np.float32)


def run_debug(upto, **inputs):
    key, in_maps = _prep_inputs(inputs)
    nc = _build(*key, upto=upto)
    res = bass_utils.run_bass_kernel_spmd(nc, in_maps, core_ids=list(range(NCORES)))
    return key, res


# revision 10
# speedup vs baseline: 1.8556x; 1.1485x over previous
"""Trainium2 Bass kernel for nn_NNModel2 (2x NNConv GNN + pooled MLP readout).

Self-contained: accepts FULL inputs, returns the FULL [256, 1] output.

Sharding: one balanced node-ownership map (512 nodes/core, chosen so that both
per-core edge counts are ~1024). conv1 uses dst-sharded edges with host-staged
x[src] tables; conv2 uses src-sharded edges so h1[src] is a local device
gather. conv2's output feeds only the linear graph mean-pool, so its messages
are scattered directly into transposed per-graph partial sums; lin1 is applied
to the (rec-scaled) partials before the collective, so a single bf16 AllReduce
of [128, 256] combines cores. No AllGather anywhere.

Math (per NNConv layer, aggr='add'):
    w_e  = (edge_attr @ nn_w + nn_b).reshape(E, I, O)
    msg  = einsum('ei,eio->eo', x[src], w_e)
restructured as a dense matmul over z[e,(k,i)] = attr[e,k] * x[src[e],i]:
    msg = z @ W' + x[src] @ B'
Scatter-add (nodes for conv1, graphs for conv2) via one-hot matmuls.
"""

import sys

sys.path.insert(0, "/opt/trn_rl_repo")

import numpy as np

from concourse import bacc, bass, mybir
import concourse.tile as tile
from concourse import bass_utils

P = 128
NCORES = 8
N_NODES = 4096
N_EDGES = 8192
N_GRAPHS = 256
DN = 64
DE = 32
H = 256
NSH = N_NODES // NCORES  # 512
NT = NSH // P  # 4
GT = N_GRAPHS // P  # 2

F32 = mybir.dt.float32
BF16 = mybir.dt.bfloat16
F16 = mybir.dt.float16
I16 = mybir.dt.int16
AF = mybir.ActivationFunctionType
ALU = mybir.AluOpType

_cache = {}


def _wrap_idx(idx, n):
    idx = np.asarray(idx, dtype=np.int16)
    assert idx.shape == (n,) and n % 16 == 0
    return np.tile(idx.reshape(n // 16, 16).T, (8, 1)).copy()


def _build(e_padA, e_padB, upto="full"):
    ETA = e_padA // P
    ETB = e_padB // P
    nc = bacc.Bacc(num_devices=NCORES)

    # ---- external inputs (host-prepped layouts, bf16 where possible)
    xsrcT_in = nc.dram_tensor("xsrcT_in", [P, e_padA], BF16, kind="ExternalInput")
    xshT_in = nc.dram_tensor("xshT_in", [DN, NSH], BF16, kind="ExternalInput")
    attrAe_in = nc.dram_tensor("attrAe_in", [16, e_padA], BF16, kind="ExternalInput")
    attrAo_in = nc.dram_tensor("attrAo_in", [16, e_padA], BF16, kind="ExternalInput")
    attrB_in = nc.dram_tensor("attrB_in", [DE, e_padB], BF16, kind="ExternalInput")
    w1p = nc.dram_tensor("w1p", [P, 16, H], BF16, kind="ExternalInput")
    b1p_in = nc.dram_tensor("b1p_in", [DN, H], BF16, kind="ExternalInput")
    r1w_in = nc.dram_tensor("r1w_in", [DN, H], BF16, kind="ExternalInput")
    b1row_in = nc.dram_tensor("b1row_in", [1, H], BF16, kind="ExternalInput")
    w2p = nc.dram_tensor("w2p", [P, 64, H], BF16, kind="ExternalInput")
    b2p_in = nc.dram_tensor("b2p_in", [P, 2, H], BF16, kind="ExternalInput")
    r2w_in = nc.dram_tensor("r2w_in", [P, 2, H], BF16, kind="ExternalInput")
    b2colT_in = nc.dram_tensor("b2colT_in", [P, 2, 1], BF16, kind="ExternalInput")
    l1w_in = nc.dram_tensor("l1w_in", [P, 2, H // 2], BF16, kind="ExternalInput")
    l1b_in = nc.dram_tensor("l1b_in", [H // 2, 1], F32, kind="ExternalInput")
    l2w_in = nc.dram_tensor("l2w_in", [H // 2, 1], BF16, kind="ExternalInput")
    l2b_in = nc.dram_tensor("l2b_in", [1, 1], F32, kind="ExternalInput")
    io512_in = nc.dram_tensor("io512_in", [P, NSH], F16, kind="ExternalInput")
    iotag_in = nc.dram_tensor("iotag_in", [P, N_GRAPHS], F16, kind="ExternalInput")
    recrow_in = nc.dram_tensor("recrow_in", [1, N_GRAPHS], BF16, kind="ExternalInput")
    mask_in = nc.dram_tensor("mask_in", [1, N_GRAPHS], BF16, kind="ExternalInput")
    # per-core index tables
    srcB_w = nc.dram_tensor("srcB_w", [P, e_padB // 16], I16, kind="ExternalInput")
    id512_w = nc.dram_tensor("id512_w", [P, NSH // 16], I16, kind="ExternalInput")
    dstlA_in = nc.dram_tensor("dstlA_in", [e_padA, 1], F32, kind="ExternalInput")
    gdstB_in = nc.dram_tensor("gdstB_in", [e_padB, 1], F32, kind="ExternalInput")
    batchl_in = nc.dram_tensor("batchl_in", [NSH, 1], F32, kind="ExternalInput")
    out = nc.dram_tensor("out", [N_GRAPHS, 1], F32, kind="ExternalOutput")

    def dbg_out(name, shape):
        return nc.dram_tensor(name, shape, F32, kind="ExternalOutput")

    rg = [list(range(NCORES))]
    ST = {"gather": 1, "msg1": 2, "h1": 3, "msg2": 4, "pool": 5, "full": 99}[upto]

    with tile.TileContext(nc, num_cores=NCORES) as tc:
        with (
            tc.tile_pool(name="const", bufs=1) as cp,
            tc.tile_pool(name="work", bufs=3) as wp,
            tc.tile_pool(name="dram", bufs=1, space="DRAM") as dr,
        ):
            # ======== stage 0: conv1-critical loads first
            xsrcT = cp.tile([P, e_padA], BF16)
            nc.sync.dma_start(out=xsrcT[:], in_=xsrcT_in[:])
            bcA = cp.tile([P, 16, e_padA], BF16)
            for c in range(2):
                sl = slice(8 * c, 8 * (c + 1))
                nc.sync.dma_start(
                    out=bcA[0:DN, sl, :],
                    in_=attrAe_in[sl, :].partition_broadcast(DN),
                )
                nc.sync.dma_start(
                    out=bcA[DN:P, sl, :],
                    in_=attrAo_in[sl, :].partition_broadcast(DN),
                )
            w1sb = cp.tile([P, 16, H], BF16)
            nc.sync.dma_start(out=w1sb[:], in_=w1p[:])
            b1p = cp.tile([DN, H], BF16)
            nc.sync.dma_start(out=b1p[:], in_=b1p_in[:])
            r1wb = cp.tile([DN, H], BF16)
            nc.sync.dma_start(out=r1wb[:], in_=r1w_in[:])
            b1row = cp.tile([1, H], BF16)
            nc.sync.dma_start(out=b1row[:], in_=b1row_in[:])
            io512 = cp.tile([P, NSH], F16)
            nc.sync.dma_start(out=io512[:], in_=io512_in[:])
            dstlA_sb = cp.tile([P, ETA, 1], F32)
            nc.sync.dma_start(
                out=dstlA_sb[:], in_=dstlA_in.rearrange("(e p) one -> p e one", p=P)
            )
            xshT = cp.tile([DN, NSH], BF16)
            nc.sync.dma_start(out=xshT[:], in_=xshT_in[:])
            ones = cp.tile([1, P], BF16)
            nc.vector.memset(ones[:], 1.0)

            # ---- conv2 weights + broadcast: stream during conv1
            srcB_sb = cp.tile([P, e_padB // 16], I16)
            nc.sync.dma_start(out=srcB_sb[:], in_=srcB_w[:])
            id512_sb = cp.tile([P, NSH // 16], I16)
            nc.sync.dma_start(out=id512_sb[:], in_=id512_w[:])
            w2sb = cp.tile([P, 64, H], BF16)
            bcB = cp.tile([P, DE, e_padB], BF16)
            nc.sync.dma_start(
                out=bcB[:, 0:8, :], in_=attrB_in[0:8, :].partition_broadcast(P)
            )
            nc.sync.dma_start(out=w2sb[:, 0:16, :], in_=w2p[:, 0:16, :])
            for c in range(1, 4):
                nc.sync.dma_start(
                    out=bcB[:, 8 * c : 8 * (c + 1), :],
                    in_=attrB_in[8 * c : 8 * (c + 1), :].partition_broadcast(P),
                )
                nc.sync.dma_start(
                    out=w2sb[:, 16 * c : 16 * (c + 1), :],
                    in_=w2p[:, 16 * c : 16 * (c + 1), :],
                )
            # small consts
            iotag = cp.tile([P, N_GRAPHS], F16)
            nc.sync.dma_start(out=iotag[:], in_=iotag_in[:])
            gdstl_sb = cp.tile([P, ETB, 1], F32)
            nc.sync.dma_start(
                out=gdstl_sb[:], in_=gdstB_in.rearrange("(e p) one -> p e one", p=P)
            )
            batchl_sb = cp.tile([P, NT, 1], F32)
            nc.sync.dma_start(
                out=batchl_sb[:], in_=batchl_in.rearrange("(t p) one -> p t one", p=P)
            )
            r2wb = cp.tile([P, 2, H], BF16)
            nc.sync.dma_start(out=r2wb[:], in_=r2w_in[:])
            b2p = cp.tile([P, 2, H], BF16)
            nc.sync.dma_start(out=b2p[:], in_=b2p_in[:])
            b2colT = cp.tile([P, 2, 1], BF16)
            nc.sync.dma_start(out=b2colT[:], in_=b2colT_in[:])
            l1wb = cp.tile([P, 2, H // 2], BF16)
            nc.sync.dma_start(out=l1wb[:], in_=l1w_in[:])
            l1bsb = cp.tile([H // 2, 1], F32)
            nc.sync.dma_start(out=l1bsb[:], in_=l1b_in[:])
            l2wb = cp.tile([H // 2, 1], BF16)
            nc.sync.dma_start(out=l2wb[:], in_=l2w_in[:])
            l2bsb = cp.tile([1, 1], F32)
            nc.sync.dma_start(out=l2bsb[:], in_=l2b_in[:])
            recbc = cp.tile([P, N_GRAPHS], BF16)
            nc.sync.dma_start(
                out=recbc[:], in_=recrow_in[0:1, :].partition_broadcast(P)
            )
            masksb = cp.tile([1, N_GRAPHS], BF16)
            nc.sync.dma_start(out=masksb[:], in_=mask_in[:])

            if upto == "gather":
                d = dbg_out("d_xsrcT", [P, e_padA])
                tmp = wp.tile([P, e_padA], F32, tag="dbgf")
                nc.vector.tensor_copy(out=tmp[:], in_=xsrcT[:])
                nc.sync.dma_start(out=d[:], in_=tmp[:])
                d2 = dbg_out("d_bcB5", [P, e_padB])
                tmp2 = wp.tile([P, e_padB], F32, tag="dbgf")
                nc.vector.tensor_copy(out=tmp2[:], in_=bcB[:, 5, :])
                nc.sync.dma_start(out=d2[:], in_=tmp2[:])

            with tc.tile_pool(name="psA", bufs=1, space="PSUM") as psA:
                # ======== conv1: msg1 = z1 @ W1' + x_src @ B1'
                msg_ps = [
                    psA.tile([P, 2 * H], F32, space="PSUM",
                             tag=f"msg{j}", name=f"msg1_{j}")
                    for j in range((ETA + 1) // 2)
                ]

                def m1(e):
                    return msg_ps[e // 2][:, (e % 2) * H : (e % 2) * H + H]

                if ST >= 2:
                    for t in range(16):
                        zt = wp.tile([P, e_padA], BF16, tag="zt", bufs=4)
                        nc.vector.tensor_tensor(
                            out=zt[:], in0=xsrcT[:], in1=bcA[:, t, :], op=ALU.mult
                        )
                        for e in range(ETA):
                            nc.tensor.matmul(
                                m1(e), lhsT=zt[:, P * e : P * (e + 1)],
                                rhs=w1sb[:, t, :],
                                start=(t == 0 and e % 2 == 0), stop=False,
                                skip_group_check=True,
                            )
                    for e in range(ETA):
                        nc.tensor.matmul(
                            m1(e), lhsT=xsrcT[0:DN, P * e : P * (e + 1)],
                            rhs=b1p[:], start=False, stop=True,
                            skip_group_check=True,
                        )

                    msbs = []
                    for j in range((ETA + 1) // 2):
                        w = min(2 * H, (ETA - 2 * j) * H)
                        msb = wp.tile([P, 2 * H], BF16, tag="msb")
                        nc.scalar.activation(
                            out=msb[:, 0:w], in_=msg_ps[j][:, 0:w], func=AF.Copy
                        )
                        msbs.append(msb)

                if upto == "msg1":
                    dm = dbg_out("d_msg1", [P, ETA * H])
                    for j in range((ETA + 1) // 2):
                        w = min(2 * H, (ETA - 2 * j) * H)
                        tmpm = wp.tile([P, 2 * H], F32, tag="dbgm")
                        nc.vector.tensor_copy(out=tmpm[:, 0:w], in_=msbs[j][:, 0:w])
                        nc.sync.dma_start(
                            out=dm[:, 2 * H * j : 2 * H * j + w], in_=tmpm[:, 0:w])

                # ---- conv1 scatter to own nodes + root + bias, relu
                agg_ps = [
                    psA.tile([P, 2 * H], F32, space="PSUM",
                             tag=f"agg{j}", name=f"agg1_{j}")
                    for j in range(NT // 2)
                ]

                def a1(n):
                    return agg_ps[n // 2][:, (n % 2) * H : (n % 2) * H + H]

                if ST >= 3:
                    for e in range(ETA):
                        for n in range(NT):
                            oh = wp.tile([P, P], BF16, tag="oh", bufs=6)
                            nc.vector.tensor_scalar(
                                out=oh[:], in0=io512[:, P * n : P * (n + 1)],
                                scalar1=dstlA_sb[:, e, :1], scalar2=None,
                                op0=ALU.is_equal,
                            )
                            nc.tensor.matmul(
                                a1(n), lhsT=oh[:],
                                rhs=msbs[e // 2][:, (e % 2) * H : (e % 2) * H + H],
                                start=(e == 0 and n % 2 == 0), stop=False,
                                skip_group_check=True,
                            )
                    for n in range(NT):
                        nc.tensor.matmul(
                            a1(n), lhsT=xshT[:, P * n : P * (n + 1)],
                            rhs=r1wb[:], start=False, stop=False,
                            skip_group_check=True,
                        )
                        nc.tensor.matmul(
                            a1(n), lhsT=ones[:], rhs=b1row[:],
                            start=False, stop=True, skip_group_check=True,
                        )
                    h1sb = cp.tile([P, NT, H], BF16)
                    for j in range(NT // 2):
                        nc.scalar.activation(
                            out=h1sb[:, 2 * j : 2 * j + 2, :],
                            in_=agg_ps[j][:, 0 : 2 * H], func=AF.Relu,
                        )
                    h1loc = dr.tile([NSH, H], BF16)
                    nc.sync.dma_start(
                        out=h1loc[:].rearrange("(t p) o -> p t o", p=P), in_=h1sb[:]
                    )

                    # local gathers for conv2
                    h1srcT = cp.tile([P, 2, e_padB], BF16)
                    nc.gpsimd.dma_gather(
                        out_ap=h1srcT[:], in_ap=h1loc[:], idxs_ap=srcB_sb[:],
                        num_idxs=e_padB, num_idxs_reg=e_padB, elem_size=H,
                        transpose=True, single_packet=False,
                    )
                    h1ownT = cp.tile([P, 2, NSH], BF16)
                    nc.gpsimd.dma_gather(
                        out_ap=h1ownT[:], in_ap=h1loc[:], idxs_ap=id512_sb[:],
                        num_idxs=NSH, num_idxs_reg=NSH, elem_size=H,
                        transpose=True, single_packet=False,
                    )

                if upto == "h1":
                    dh = dbg_out("d_h1", [P, NT * H])
                    tmp = wp.tile([P, NT, H], F32, tag="dbgf")
                    nc.vector.tensor_copy(out=tmp[:], in_=h1sb[:])
                    nc.sync.dma_start(
                        out=dh[:].rearrange("p (t o) -> p t o", o=H), in_=tmp[:]
                    )

                # ---- early small matmul: b2l1 = (b2/8) @ l1w (for bias2 fold)
                with tc.tile_pool(name="psS", bufs=1, space="PSUM") as psS:
                    b2l1_ps = psS.tile([1, H // 2], F32, space="PSUM", tag="b2l1")
                    for kh in range(2):
                        nc.tensor.matmul(
                            b2l1_ps[:], lhsT=b2colT[:, kh, :1], rhs=l1wb[:, kh, :],
                            start=(kh == 0), stop=(kh == 1),
                        )
                    b2l1sb = cp.tile([1, H // 2], BF16)
                    nc.scalar.activation(out=b2l1sb[:], in_=b2l1_ps[:], func=AF.Copy)

                # ======== conv2: msg2 = z2 @ W2' + h1_src @ B2'
                if ST >= 4:
                    msg2_ps = [
                        psA.tile([P, 2 * H], F32, space="PSUM",
                                 tag=f"msg{j}", name=f"msg2_{j}")
                        for j in range((ETB + 1) // 2)
                    ]

                    def m2(e):
                        return msg2_ps[e // 2][:, (e % 2) * H : (e % 2) * H + H]

                    for t in range(64):
                        k, ih = t // 2, t % 2
                        zt = wp.tile([P, e_padB], BF16, tag="zt", bufs=4)
                        nc.vector.tensor_tensor(
                            out=zt[:], in0=h1srcT[:, ih, :], in1=bcB[:, k, :],
                            op=ALU.mult,
                        )
                        for e in range(ETB):
                            nc.tensor.matmul(
                                m2(e), lhsT=zt[:, P * e : P * (e + 1)],
                                rhs=w2sb[:, t, :],
                                start=(t == 0 and e % 2 == 0), stop=False,
                                skip_group_check=True,
                            )
                    for e in range(ETB):
                        for ih in range(2):
                            nc.tensor.matmul(
                                m2(e), lhsT=h1srcT[:, ih, P * e : P * (e + 1)],
                                rhs=b2p[:, ih, :], start=False,
                                stop=(ih == 1), skip_group_check=True,
                            )

                    msbs2 = []
                    for j in range((ETB + 1) // 2):
                        w = min(2 * H, (ETB - 2 * j) * H)
                        msb = wp.tile([P, 2 * H], BF16, tag="msb")
                        nc.scalar.activation(
                            out=msb[:, 0:w], in_=msg2_ps[j][:, 0:w], func=AF.Copy
                        )
                        msbs2.append(msb)

                if upto == "msg2":
                    dm = dbg_out("d_msg2", [P, ETB * H])
                    for j in range((ETB + 1) // 2):
                        w = min(2 * H, (ETB - 2 * j) * H)
                        tmpm = wp.tile([P, 2 * H], F32, tag="dbgm")
                        nc.vector.tensor_copy(out=tmpm[:, 0:w], in_=msbs2[j][:, 0:w])
                        nc.sync.dma_start(
                            out=dm[:, 2 * H * j : 2 * H * j + w], in_=tmpm[:, 0:w])

                # ---- conv2 root transform r2 = h1_own @ root2_w
                if ST >= 4:
                    r2_ps = [
                        psA.tile([P, 2 * H], F32, space="PSUM",
                                 tag=f"agg{j}", name=f"r2_{j}")
                        for j in range(NT // 2)
                    ]
                    for n in range(NT):
                        for kh in range(2):
                            nc.tensor.matmul(
                                r2_ps[n // 2][:, (n % 2) * H : (n % 2) * H + H],
                                lhsT=h1ownT[:, kh, P * n : P * (n + 1)],
                                rhs=r2wb[:, kh, :],
                                start=(kh == 0), stop=(kh == 1),
                                skip_group_check=True,
                            )
                    r2sb = cp.tile([P, NT, H], BF16)
                    for j in range(NT // 2):
                        nc.scalar.activation(
                            out=r2sb[:, 2 * j : 2 * j + 2, :],
                            in_=r2_ps[j][:, 0 : 2 * H], func=AF.Copy,
                        )

                    # ---- transposed scatter into poolT[o, g]:
                    #   poolT[o, g] += sum_e msg2[e, o]*[gdst(e)=g]
                    #                + sum_n r2[n, o]*[batch(n)=g]
                    pool_t = psA.tile([P, 2 * H], F32, space="PSUM",
                                      tag="pool", name="poolT")
                    for e in range(ETB):
                        ohg = wp.tile([P, N_GRAPHS], BF16, tag="ohg", bufs=4)
                        nc.vector.tensor_scalar(
                            out=ohg[:], in0=iotag[:],
                            scalar1=gdstl_sb[:, e, :1], scalar2=None,
                            op0=ALU.is_equal,
                        )
                        for hh in range(2):
                            nc.tensor.matmul(
                                pool_t[:, hh * H : hh * H + H],
                                lhsT=msbs2[e // 2][
                                    :, (e % 2) * H + hh * P : (e % 2) * H + hh * P + P
                                ],
                                rhs=ohg[:],
                                start=(e == 0 and hh == 0), stop=False,
                                skip_group_check=True,
                            )
                    for n in range(NT):
                        ohb = wp.tile([P, N_GRAPHS], BF16, tag="ohg", bufs=4)
                        nc.vector.tensor_scalar(
                            out=ohb[:], in0=iotag[:],
                            scalar1=batchl_sb[:, n, :1], scalar2=None,
                            op0=ALU.is_equal,
                        )
                        for hh in range(2):
                            nc.tensor.matmul(
                                pool_t[:, hh * H : hh * H + H],
                                lhsT=r2sb[:, n, hh * P : hh * P + P],
                                rhs=ohb[:],
                                start=False, stop=(n == NT - 1 and hh == 1),
                                skip_group_check=True,
                            )
                    # poolT psum -> bf16, scale by 1/cnt (per graph, free axis)
                    plsb = cp.tile([P, 2, N_GRAPHS], BF16)
                    nc.scalar.activation(
                        out=plsb[:], in_=pool_t[:, 0 : 2 * H], func=AF.Copy
                    )
                    scl = cp.tile([P, 2, N_GRAPHS], BF16)
                    for hh in range(2):
                        nc.vector.tensor_tensor(
                            out=scl[:, hh, :], in0=plsb[:, hh, :], in1=recbc[:],
                            op=ALU.mult,
                        )
                    # z1 partial = scl^T @ l1w + (b2/8 @ l1w) x mask
                    z1p_t = psA.tile([P, 2 * H], F32, space="PSUM",
                                     tag="pool", name="z1p")
                    z1p = z1p_t[:, 0:N_GRAPHS]
                    for hh in range(2):
                        nc.tensor.matmul(
                            z1p, lhsT=l1wb[:, hh, :], rhs=scl[:, hh, :],
                            start=(hh == 0), stop=False, skip_group_check=True,
                        )
                    nc.tensor.matmul(
                        z1p, lhsT=b2l1sb[:], rhs=masksb[:],
                        start=False, stop=True, skip_group_check=True,
                    )
                    z1psb = cp.tile([H // 2, N_GRAPHS], BF16)
                    nc.scalar.activation(out=z1psb[:], in_=z1p, func=AF.Copy)
                    pcc_in = dr.tile([H // 2, N_GRAPHS], BF16)
                    nc.sync.dma_start(out=pcc_in[:], in_=z1psb[:])
                    pcc_out = dr.tile([H // 2, N_GRAPHS], BF16, addr_space="Shared")
                    nc.gpsimd.collective_compute(
                        "AllReduce", ALU.add, replica_groups=rg,
                        ins=[pcc_in[:].opt()], outs=[pcc_out[:].opt()],
                    )

                if upto == "pool":
                    dr2 = dbg_out("d_r2", [P, NT * H])
                    tmp2 = wp.tile([P, NT, H], F32, tag="dbgr")
                    nc.vector.tensor_copy(out=tmp2[:], in_=r2sb[:])
                    nc.sync.dma_start(
                        out=dr2[:].rearrange("p (t o) -> p t o", o=H), in_=tmp2[:]
                    )
                    dp = dbg_out("d_poolT", [P, 2 * N_GRAPHS])
                    tmp = wp.tile([P, 2, N_GRAPHS], F32, tag="dbgf")
                    nc.vector.tensor_copy(out=tmp[:], in_=plsb[:])
                    nc.sync.dma_start(
                        out=dp[:].rearrange("p (h g) -> p h g", g=N_GRAPHS), in_=tmp[:]
                    )
                    dz = dbg_out("d_z1p", [H // 2, N_GRAPHS])
                    tmpz = wp.tile([H // 2, N_GRAPHS], F32, tag="dbgz")
                    nc.vector.tensor_copy(out=tmpz[:], in_=z1psb[:])
                    nc.sync.dma_start(out=dz[:], in_=tmpz[:])

            # ======== tail: readout MLP (redundant on every core)
            if ST >= 5 and upto == "full":
                with tc.tile_pool(name="psB", bufs=1, space="PSUM") as psB:
                    pl = cp.tile([H // 2, N_GRAPHS], BF16)
                    nc.sync.dma_start(out=pl[:], in_=pcc_out[:])
                    z1sb = cp.tile([H // 2, N_GRAPHS], BF16)
                    nc.scalar.activation(
                        out=z1sb[:], in_=pl[:], func=AF.Relu, bias=l1bsb[:, :1]
                    )
                    o_ps = psB.tile([1, N_GRAPHS], F32, space="PSUM", tag="op")
                    nc.tensor.matmul(
                        o_ps[:], lhsT=l2wb[:], rhs=z1sb[:], start=True, stop=True
                    )
                    osb = cp.tile([1, N_GRAPHS], F32)
                    nc.scalar.activation(
                        out=osb[:], in_=o_ps[:], func=AF.Sigmoid, bias=l2bsb[:, :1]
                    )
                    nc.sync.dma_start(
                        out=out[:].rearrange("g one -> one g"), in_=osb[:]
                    )

    nc.compile()
    return nc


def _balance_owner(src, dst):
    """Assign 512 nodes/core s.t. per-core indeg and outdeg sums are balanced."""
    indeg = np.bincount(dst, minlength=N_NODES)
    outdeg = np.bincount(src, minlength=N_NODES)
    order = np.argsort(-(indeg + outdeg), kind="stable")
    owner = np.full(N_NODES, -1, dtype=np.int64)
    in_load = np.zeros(NCORES, dtype=np.int64)
    out_load = np.zeros(NCORES, dtype=np.int64)
    slots = np.full(NCORES, NSH, dtype=np.int64)
    for n in order:
        best, bkey = -1, None
        for c in range(NCORES):
            if slots[c] == 0:
                continue
            key = (
                max(in_load[c] + indeg[n], out_load[c] + outdeg[n]),
                in_load[c] + out_load[c],
            )
            if bkey is None or key < bkey:
                best, bkey = c, key
        owner[n] = best
        in_load[best] += indeg[n]
        out_load[best] += outdeg[n]
        slots[best] -= 1
    return owner, int(in_load.max()), int(out_load.max())


def _bf16(a):
    import ml_dtypes

    return np.asarray(np.asarray(a, np.float32), dtype=ml_dtypes.bfloat16)


def _prep_inputs(inputs):
    x = np.asarray(inputs["x"], dtype=np.float32)
    ei = np.asarray(inputs["edge_index"])
    attr = np.asarray(inputs["edge_attr"], dtype=np.float32)
    batch = np.asarray(inputs["batch"]).astype(np.int64)
    src, dst = ei[0].astype(np.int64), ei[1].astype(np.int64)

    owner, max_in, max_out = _balance_owner(src, dst)
    e_padA = max(((max_in + P - 1) // P) * P, P)
    e_padB = max(((max_out + P - 1) // P) * P, P)

    own = [np.nonzero(owner == c)[0] for c in range(NCORES)]
    local_id = np.zeros(N_NODES, dtype=np.int64)
    for c in range(NCORES):
        local_id[own[c]] = np.arange(NSH)

    nn1_w = np.asarray(inputs["nn1_w"], dtype=np.float32)  # [32, 64*256]
    nn2_w = np.asarray(inputs["nn2_w"], dtype=np.float32)  # [32, 256*256]
    nn1_b = np.asarray(inputs["nn1_b"], dtype=np.float32)
    nn2_b = np.asarray(inputs["nn2_b"], dtype=np.float32)

    # w1p[p, t, o] = nn1_w[2t + p//64, (p%64)*256 + o]
    w1r = nn1_w.reshape(16, 2, DN, H)  # [t, k2, i, o]
    w1p = np.ascontiguousarray(w1r.transpose(1, 2, 0, 3).reshape(P, 16, H))
    # w2p[p, t, o] = nn2_w[t//2, ((t%2)*128 + p)*256 + o]
    w2r = nn2_w.reshape(DE, 2, P, H)  # [k, ih, p, o]
    w2p = np.ascontiguousarray(w2r.transpose(2, 0, 1, 3).reshape(P, 64, H))
    b2pr = nn2_b.reshape(2, P, H).transpose(1, 0, 2)  # [p, ih, o]

    cnt = np.bincount(batch, minlength=N_GRAPHS).astype(np.float32)
    recrow = (1.0 / np.maximum(cnt, 1.0)).reshape(1, N_GRAPHS)
    maskrow = (cnt > 0).astype(np.float32).reshape(1, N_GRAPHS)

    r2w = np.asarray(inputs["root2_w"], dtype=np.float32)  # [256, 256]
    b2 = np.asarray(inputs["bias2"], dtype=np.float32)  # [256]
    l1w = np.asarray(inputs["lin1_w"], dtype=np.float32)  # [256, 128]

    x_bf = _bf16(x)
    attr_bf = _bf16(attr)

    common = {
        "w1p": _bf16(w1p),
        "b1p_in": _bf16(nn1_b.reshape(DN, H)),
        "r1w_in": _bf16(np.asarray(inputs["root1_w"], np.float32)),
        "b1row_in": _bf16(np.asarray(inputs["bias1"], np.float32).reshape(1, H)),
        "w2p": _bf16(w2p),
        "b2p_in": _bf16(b2pr),
        "r2w_in": _bf16(r2w.reshape(2, P, H).transpose(1, 0, 2)),
        "b2colT_in": _bf16((b2 / NCORES).reshape(2, P, 1).transpose(1, 0, 2)),
        "l1w_in": _bf16(l1w.reshape(2, P, H // 2).transpose(1, 0, 2)),
        "l1b_in": np.asarray(inputs["lin1_b"], np.float32).reshape(-1, 1),
        "l2w_in": _bf16(np.asarray(inputs["lin2_w"], np.float32)),
        "l2b_in": np.asarray(inputs["lin2_b"], np.float32).reshape(1, 1),
        "io512_in": np.tile(np.arange(NSH, dtype=np.float16), (P, 1)),
        "iotag_in": np.tile(np.arange(N_GRAPHS, dtype=np.float16), (P, 1)),
        "recrow_in": _bf16(recrow),
        "mask_in": _bf16(maskrow),
        "id512_w": _wrap_idx(np.arange(NSH, dtype=np.int16), NSH),
    }

    in_maps = []
    for c in range(NCORES):
        eA = np.nonzero(owner[dst] == c)[0]
        eB = np.nonzero(owner[src] == c)[0]
        nA, nB = len(eA), len(eB)
        assert nA <= e_padA and nB <= e_padB

        # host-staged gather tables for conv1 (bf16)
        xsrcA = np.zeros((P, e_padA), dtype=x_bf.dtype)
        xsrcA[0:DN, :nA] = x_bf[src[eA]].T
        xsrcA[DN:P, :nA] = x_bf[src[eA]].T
        attrA = np.zeros((DE, e_padA), dtype=attr_bf.dtype)
        attrA[:, :nA] = attr_bf[eA].T
        attrB = np.zeros((DE, e_padB), dtype=attr_bf.dtype)
        attrB[:, :nB] = attr_bf[eB].T
        xsh = np.ascontiguousarray(x_bf[own[c]].T)  # [64, 512]

        dstlA = np.full(e_padA, -1.0, dtype=np.float32)
        dstlA[:nA] = local_id[dst[eA]].astype(np.float32)
        srcB = np.zeros(e_padB, dtype=np.int16)
        srcB[:nB] = local_id[src[eB]]
        gdstB = np.full(e_padB, -1.0, dtype=np.float32)
        gdstB[:nB] = batch[dst[eB]].astype(np.float32)

        m = dict(common)
        m["xsrcT_in"] = xsrcA
        m["xshT_in"] = xsh
        m["attrAe_in"] = np.ascontiguousarray(attrA[0::2, :])
        m["attrAo_in"] = np.ascontiguousarray(attrA[1::2, :])
        m["attrB_in"] = attrB
        m["srcB_w"] = _wrap_idx(srcB, e_padB)
        m["dstlA_in"] = dstlA.reshape(-1, 1)
        m["gdstB_in"] = gdstB.reshape(-1, 1)
        m["batchl_in"] = batch[own[c]].astype(np.float32).reshape(-1, 1)
        in_maps.append(m)
    return (e_padA, e_padB), in_maps


def kernel(**inputs) -> np.ndarray:
    key, in_maps = _prep_inputs(inputs)
    if key not in _cache:
        _cache[key] = _build(*key)
    nc = _cache[key]
    res = bass_utils.run_bass_kernel_spmd(nc, in_maps, core_ids=list(range(NCORES)))
    return np.asarray(res.results[0]["out"], dtype=np.float32)


def run_debug(upto, **inputs):
    key, in_maps = _prep_inputs(inputs)
    nc = _build(*key, upto=upto)
    res = bass_utils.run_bass_kernel_spmd(nc, in_maps, core_ids=list(range(NCORES)))
    return key, res
